# revision 46
# baseline (speedup 1.0000x reference)
"""ALNN layer kernel for 8 TRN2 NeuronCores (raw Bass, explicit semaphores).

out[b,r,d] = relu( sum_l w_v[r,l,d]*relu(z[b,r,l,d]) + L*b_v[r,d] )
z = wt0*X + wt1*relu(X)*k + wt2*M + wt3*PD + 4*bt
k = exp(-relu(alpha_r)*|T - s_r|)        (uses relu(X*k) == relu(X)*k, k>0)

Sharding: B split 2 ways x R dealt into 4 buckets -> 8 cores, 16 b x 12 r
each. Cores c and c+4 share bucket c%4. r's with relu(alpha)=0 take a
fast path (k == 1). The graph is SPMD-uniform: a shared fast-mask sized
by the min zero-count across buckets.

v8 (from v7 @ 60816ns -> ~56.5us measured): DVE-bound design.
 - DVE is the saturated engine (~40.5us): products merged into ONE
   quad op [q|m0|m2|m3] per fast iter (FD=4096 @2x_1p; tensor_tensor
   is capped at 2x — no 4x uop) or tri+g+qs (slow). wl = lat*wv also
   on DVE (gpsimd tensor ops are ~2x slower than modeled AND slow
   concurrent DVE ops ~4x via SBUF contention — do not use).
 - Slow slots interleaved at 3,5,7,9 (SLOW_FILL) so ACT-heavy abs/exp
   iters alternate with DVE-heavy slow-product iters; abs/exp pair j
   emitted at ACT iter slows[j]-1.
 - PE: z via identity matmuls (one vsem wait per DVE op), bt via
   btsl x OHD trick, out-reduce via one-hot; ~14 warmup matmuls hold
   the p-state until real work. Every psum column's first write needs
   start=True; reading a psum bank mid-accumulation-group faults HW.
 - ACT: relu + slow-iter abs/exp; a dummy activation right after the
   cfc wait forces the lazy ~1.3us ACT_TABLE_LOAD off the critical
   path. Final relus read ps0/ps1 only after msem 12/13 (groups
   closed); output as bf16, column-split DMAs from the sync ring.
 - DMA: ~7.3us fixed framework preamble, then ~230-260GB/s aggregate
   over 16 engines shared by all rings. Consumption-ordered chunks:
   sync ring [Xt-h0 | W0W1 | Xt-h1], deferred [W2-5 | W6-11] after
   s3; ACT ring cfc, CST, Mt, PDt; gpsimd SWDGE ring OBA, then
   (deferred) cfT, OBB. Same-ring DMA completion is FIFO per engine,
   so a later chunk's semaphore implies earlier chunks landed.
 - Tail: last iter's quad split into column halves feeding the PE's
   half-closed (z2sem) last z-group; wl via scalar_tensor_tensor
   straight from PSUM; NaN-retry guard for a rare (~1/20) transient.

Raw bass: at most ONE attached sync-wait per compute instruction, so
cross-engine deps use standalone wait_ge; each DMA gets a dedicated
semaphore (two DMAs sharing one sem can interleave per-queue completions,
so a partial wait would be unsound).
"""

import os
import numpy as np
import ml_dtypes

import concourse.bass as bass
import concourse.mybir as mybir
from concourse.bass_utils import run_bass_kernel_spmd

AF = mybir.ActivationFunctionType
OP = mybir.AluOpType
BF16 = mybir.dt.bfloat16
F32 = mybir.dt.float32

B, R, L, D = 32, 48, 128, 64
NB, NK = 2, 4              # b-halves x r-buckets = 8 cores
BC, RC = B // NB, R // NK  # 16 b, 12 r per core
FD = BC * D                # 1024 free elems

CFC_W = 2 * RC             # f32 consts: [Abc 12 | NASbc 12]
WS_W = 5 * D               # per-iter param slice: [w1|w0|w2|w3|wv]
STRD = FD + 2 * WS_W       # 1664: uniform stride of the data planes

# BIG sbuf layout (bf16), stride-1664 planes:
XPC = 0                    # Xp   [0:1024]
WSCC = 1024                # wsc  [1024:1536] (warmup ones), pad to 1664
XC = STRD                  # Xt   [1664:2688] | pad [2688:3328]
MC = 2 * STRD              # Mt   [3328:4352] | pad
PDC = 3 * STRD             # PDt  [4992:6016]
W0C = PDC + FD             # W0..W11 [6016:9856]
GC = W0C + 12 * WS_W       # g    [9856:10880]
BIG_W = GC + FD            # 10880

# DRAM cb stream, consumption order:
# [Xt-h0 | W0 W1 | Xt-h1 | Mt | PDt | W2-5 | W6-11]
CB_W = 6912
XH0E = 0
W01E = 512
XH1E = W01E + 2 * WS_W     # 1152
MTE = XH1E + 512           # 1664
PDE = MTE + FD             # 2688
W25E = PDE + FD            # 3712
W611E = W25E + 4 * WS_W    # 4992

CONST_W = RC * RC + L      # [OH3 144 | Ident 128]
OBA_W = 512 + 16 + 4 * L   # [OHD | lbvT(16 pad) | BT0-3]
OBB_W = 8 * L              # [BT4-11]


def wcol(i):
    return W0C + i * WS_W

_nbf16 = ml_dtypes.bfloat16

WLPOOL = False  # gpsimd tensor ops are slow and poison concurrent DVE ops

LB = 5    # wl buffers (Pool/DVE -> PE)
LAB = 4   # lat buffers (ACT -> Pool)
PRB = 3   # product buffer sets (DVE -> PE)
ZB = 3    # psum z triple-buffer (6 of 8 banks; ps0/ps1 take the rest)
WLAG = 3  # out-mm for rr = i - WLAG emitted in PE iter i
ALAG = 2  # relu for rr = i - ALAG emitted in ACT iter i
NWARM = 14  # PE warmup matmuls (keep PE out of low p-state until work)
NSPLIT = 2  # iters with split product ops (partial-input start)
NTAIL = 1   # last iters: wl via DVE stt straight from PSUM


SLOW_FILL = (3, 5, 7, 9, 2, 4, 6, 8, 1, 10, 0, 11)


def _fast_mask(nfast):
    """Interleave slow slots (odd-first) so ACT-heavy abs/exp iterations
    alternate with DVE-heavy slow-product iterations; head and tail slots
    stay fast when possible (early start, short drain)."""
    nslow = RC - nfast
    mask = [True] * RC
    for s in SLOW_FILL[:nslow]:
        mask[s] = False
    return tuple(mask)


def _dve_schedule(mask):
    """DVE emission order; must match the @block.vector body exactly."""
    sched = []
    for i in range(RC):
        if i == 0 and mask[0] and NSPLIT > 0:
            sched.append(("p1h0", 0))
            sched.append(("p1h1", 0))
        if i < NSPLIT:
            if mask[i]:
                if i > 0:
                    sched.append(("p1", i))    # [q|m0]
            else:
                sched.append(("m0s", i))
            sched.append(("p2", i))        # [m2|m3]
            if not mask[i]:
                sched.append(("g", i))
                sched.append(("qs", i))
        else:
            if mask[i]:
                if i == RC - 1:
                    if RC - 1 >= WLAG - 1:
                        sched.append(("wl", RC - 1 - (WLAG - 1)))
                    sched.append(("qh0", i))
                    sched.append(("qh1", i))
                else:
                    sched.append(("quad", i))  # [q|m0|m2|m3]
            else:
                sched.append(("tri", i))   # [m0|m2|m3]
                sched.append(("g", i))
                sched.append(("qs", i))
        if not WLPOOL and i >= WLAG - 1 and not (
            i == RC - 1 and mask[i] and i >= NSPLIT
        ):
            rr = i - (WLAG - 1)
            if rr <= RC - NTAIL - 1:
                sched.append(("wl", rr))
    if not WLPOOL:
        for rr in range(RC - (WLAG - 1), RC - NTAIL):
            sched.append(("wl", rr))
    if NTAIL >= 2:
        sched.append(("stt", RC - 2))
    sched.append(("stta", RC - 1))
    sched.append(("sttb", RC - 1))
    return sched


def _build_graph(mask, detect_races=True):
    nslow = sum(1 for f in mask if not f)
    ksl = max(nslow, 1)
    slows = [i for i in range(RC) if not mask[i]]
    kidx = {i: j for j, i in enumerate(slows)}  # slow iter -> k slot
    sched = _dve_schedule(mask)
    C = {key: idx + 1 for idx, key in enumerate(sched)}

    nc = bass.Bass(detect_race_conditions=detect_races)
    cfc_e = nc.declare_dram_parameter("cfc", [L, CFC_W], F32, isOutput=False)
    cft_e = nc.declare_dram_parameter("cfT", [L, FD], BF16, isOutput=False)
    cb_e = nc.declare_dram_parameter("cb", [L, CB_W], BF16, isOutput=False)
    cst_e = nc.declare_dram_parameter("CST", [L, CONST_W], BF16, isOutput=False)
    oba_e = nc.declare_dram_parameter("OBA", [64, OBA_W], BF16, isOutput=False)
    obb_e = nc.declare_dram_parameter("OBB", [64, OBB_W], BF16, isOutput=False)
    out_e = nc.declare_dram_parameter("out", [RC, FD], BF16, isOutput=True)

    from contextlib import ExitStack

    with ExitStack() as ctx:
        e = ctx.enter_context
        cfc = e(nc.sbuf_tensor([L, CFC_W], F32))
        cft = e(nc.sbuf_tensor([L, FD], BF16))
        BIG = e(nc.sbuf_tensor([L, BIG_W], BF16))
        CST = e(nc.sbuf_tensor([L, CONST_W], BF16))
        OBA = e(nc.sbuf_tensor([64, OBA_W], BF16))
        OBB = e(nc.sbuf_tensor([64, OBB_W], BF16))
        dist = e(nc.sbuf_tensor([L, FD], F32))
        kbuf = e(nc.sbuf_tensor([L, ksl * FD], BF16))
        prods = e(nc.sbuf_tensor([L, PRB * 4 * FD], BF16))
        latb = e(nc.sbuf_tensor([L, LAB * FD], BF16))
        wlbuf = e(nc.sbuf_tensor([L, LB * FD], BF16))
        outt = e(nc.sbuf_tensor([RC, FD], BF16))
        psz = [e(nc.psum_tensor(f"psz{j}", [L, FD], F32)) for j in range(ZB)]
        ps0 = e(nc.psum_tensor([RC, 512], F32))
        ps1 = e(nc.psum_tensor([RC, 512], F32))
        s1sem = e(nc.semaphore("s1sem"))    # Xt-h0
        s1bsem = e(nc.semaphore("s1bsem"))  # Xt-h1
        swsem = e(nc.semaphore("swsem"))    # W0|W1
        s2sem = e(nc.semaphore("s2sem"))    # cb c2: Mt|W2|W3
        s3sem = e(nc.semaphore("s3sem"))    # cb c3: PDt|W4|W5
        s4sem = e(nc.semaphore("s4sem"))    # cb c4: W2-5
        s5sem = e(nc.semaphore("s5sem"))    # cb c5: W6-11
        cfcsem = e(nc.semaphore("cfcsem"))
        cstsem = e(nc.semaphore("cstsem"))  # OH3|Ident
        cftsem = e(nc.semaphore("cftsem"))
        obasem = e(nc.semaphore("obasem"))  # OHD|lbvT|BT0-3
        obbsem = e(nc.semaphore("obbsem"))  # BT4-11
        asem = e(nc.semaphore("asem"))      # ACT k completions
        lsem = e(nc.semaphore("lsem"))      # ACT lat relu completions
        psem = e(nc.semaphore("psem"))      # Pool wl completions
        zsem = e(nc.semaphore("zsem"))      # PE z-group completions (1/iter)
        z2sem = e(nc.semaphore("z2sem"))    # PE last z-group half completions
        msem = e(nc.semaphore("msem"))      # PE out-mm completions
        vsem = e(nc.semaphore("vsem"))      # DVE op completions
        osem = e(nc.semaphore("osem"))
        rsem = e(nc.semaphore("rsem"))
        gsem = e(nc.semaphore("gsem"))
        xsem = e(nc.semaphore("xsem"))  # ACT-computed Xp halves
        block = e(nc.Block())

        Abc = cfc[:, 0:RC]
        NASbc = cfc[:, RC : 2 * RC]
        Xp = BIG[:, XPC : XPC + FD]
        Xt = BIG[:, XC : XC + FD]
        Mt = BIG[:, MC : MC + FD]
        PDt = BIG[:, PDC : PDC + FD]
        gbuf = BIG[:, GC : GC + FD]
        wsc = BIG[:, WSCC : WSCC + 512]
        OH3 = CST[:, 0 : RC * RC].rearrange("p (r m) -> p r m", r=RC)
        Ident = CST[:, RC * RC : RC * RC + L]
        OHD = OBA[:, 0:512]
        lbvT = OBA[:, 512 : 512 + RC]

        def r3(ap):
            return ap.rearrange("p (b d) -> p b d", b=BC)

        def kslot(j):
            return kbuf[:, j * FD : (j + 1) * FD]

        def wbc(i, ei):
            base = wcol(i) + ei * D
            return BIG[:, base : base + D].unsqueeze(1).broadcast_to([L, BC, D])

        def wbch(i, ei):
            base = wcol(i) + ei * D
            return (
                BIG[:, base : base + D].unsqueeze(1).broadcast_to([L, BC // 2, D])
            )

        def wgrp(i, e0, n):
            """weights slots e0..e0+n-1 of iter i: [L, n, 16, 64] bcast."""
            base = wcol(i) + e0 * D
            return (
                BIG[:, base : base + n * D]
                .rearrange("p (a d) -> p a d", a=n)
                .unsqueeze(2)
                .broadcast_to([L, n, BC, D])
            )

        def dgrp(c0, n):
            """n data planes at stride STRD from col c0: [L, n, 16, 64]."""
            return (
                BIG[:, c0 : c0 + n * STRD]
                .rearrange("p (a c) -> p a c", a=n)[:, :, 0:FD]
                .rearrange("p a (b d) -> p a b d", b=BC)
            )

        def pslot(i):
            return prods[:, (i % PRB) * 4 * FD : (i % PRB + 1) * 4 * FD]

        def pview(i, e0, n):
            s = pslot(i)
            return s[:, e0 * FD : (e0 + n) * FD].rearrange(
                "p (a b d) -> p a b d", a=n, b=BC
            )

        def btsl(i):
            if i < 4:
                return OBA[:, 512 + 16 + i * L : 512 + 16 + (i + 1) * L]
            return OBB[:, (i - 4) * L : (i - 3) * L]

        def latslot(rr):
            return latb[:, (rr % LAB) * FD : (rr % LAB + 1) * FD]

        def wlslot(rr):
            return wlbuf[:, (rr % LB) * FD : (rr % LB + 1) * FD]

        @block.sync
        def _(sp):
            sp.dma_start(
                out=BIG[:, XC : XC + 512], in_=cb_e[:, XH0E : XH0E + 512]
            ).then_inc(s1sem, 16)
            sp.dma_start(
                out=BIG[:, W0C : W0C + 2 * WS_W],
                in_=cb_e[:, W01E : W01E + 2 * WS_W],
            ).then_inc(swsem, 16)
            sp.dma_start(
                out=BIG[:, XC + 512 : XC + FD], in_=cb_e[:, XH1E : XH1E + 512]
            ).then_inc(s1bsem, 16)
            sp.wait_ge(s2sem, 16)
            sp.dma_start(
                out=BIG[:, W0C + 2 * WS_W : W0C + 6 * WS_W],
                in_=cb_e[:, W25E : W25E + 4 * WS_W],
            ).then_inc(s4sem, 16)
            sp.dma_start(
                out=BIG[:, W0C + 6 * WS_W : GC], in_=cb_e[:, W611E:]
            ).then_inc(s5sem, 16)
            sp.wait_ge(rsem, 1)
            sp.dma_start(out=out_e[:, 0:512], in_=outt[:, 0:512]).then_inc(
                osem, 16
            )
            sp.wait_ge(rsem, 2)
            sp.dma_start(out=out_e[:, 512:], in_=outt[:, 512:]).then_inc(
                osem, 16
            )

        @block.scalar
        def _(act):
            act.dma_start(out=cfc[:, :], in_=cfc_e[:, :]).then_inc(cfcsem, 16)
            act.dma_start(out=CST[:, :], in_=cst_e[:, :]).then_inc(cstsem, 16)
            act.dma_start(
                out=BIG[:, MC : MC + FD], in_=cb_e[:, MTE : MTE + FD]
            ).then_inc(s2sem, 16)
            act.dma_start(
                out=BIG[:, PDC : PDC + FD], in_=cb_e[:, PDE : PDE + FD]
            ).then_inc(s3sem, 16)
            act.wait_ge(cfcsem, 16)
            # dummy op: forces the lazy ACT_TABLE_LOAD (~1.3us) to happen
            # now, while ACT is idle, instead of before the first relu
            nc.scalar.activation(dist[0:1, 0:1], cfc[0:1, 0:1], AF.Relu)
            # Xp = relu(Xt) on the idle ACT engine (frees ~0.6us of DVE)
            act.wait_ge(s1sem, 16)
            nc.scalar.activation(Xp[:, 0:512], Xt[:, 0:512], AF.Relu).then_inc(
                xsem, 1
            )
            act.wait_ge(s1bsem, 16)
            nc.scalar.activation(Xp[:, 512:FD], Xt[:, 512:FD], AF.Relu).then_inc(
                xsem, 1
            )
            pj = [max(1, s - 1) for s in slows]
            cft_waited = False
            for i in range(RC):
                if ALAG <= i and i - ALAG <= RC - NTAIL - 1:
                    rr = i - ALAG
                    act.wait_ge(zsem, rr + 1)
                    if rr >= LAB:
                        if WLPOOL:
                            act.wait_ge(psem, rr - LAB + 1)
                        else:
                            act.wait_ge(vsem, C[("wl", rr - LAB)])
                    nc.scalar.activation(
                        latslot(rr), psz[rr % ZB][:, :], AF.Relu
                    ).then_inc(lsem, 1)
                for j in range(nslow):
                    if pj[j] != i:
                        continue
                    if not cft_waited:
                        act.wait_ge(cftsem, 16)
                        cft_waited = True
                    si = slows[j]
                    nc.scalar.activation(
                        dist[:, :], cft[:, :], AF.Abs,
                        bias=NASbc[:, si : si + 1], scale=Abc[:, si : si + 1],
                    )
                    nc.scalar.activation(
                        kslot(j), dist[:, :], AF.Exp, scale=-1.0
                    ).then_inc(asem, 1)
            # drain the output in two row chunks: rows rr are final right
            # after iteration rr's out-mm (one-hot row writes)
            for rr in range(RC - ALAG, RC - NTAIL):
                act.wait_ge(zsem, rr + 1)
                act.wait_ge(vsem, C[("wl", rr - LAB)])
                nc.scalar.activation(
                    latslot(rr), psz[rr % ZB][:, :], AF.Relu
                ).then_inc(lsem, 1)
            act.wait_ge(msem, 12)
            nc.scalar.activation(outt[:, 0:512], ps0[:, :], AF.Relu).then_inc(
                rsem, 1
            )
            act.wait_ge(msem, 13)
            nc.scalar.activation(outt[:, 512:], ps1[:, :], AF.Relu).then_inc(
                rsem, 1
            )

        def halfop(i, e0, n, h, in0c):
            c0 = h * 512
            in0h = (
                BIG[:, in0c : in0c + n * STRD]
                .rearrange("p (a c) -> p a c", a=n)[:, :, c0 : c0 + 512]
                .rearrange("p a (b d) -> p a b d", b=BC // 2)
            )
            wh = (
                BIG[:, wcol(i) + e0 * D : wcol(i) + (e0 + n) * D]
                .rearrange("p (a d) -> p a d", a=n)
                .unsqueeze(2)
                .broadcast_to([L, n, BC // 2, D])
            )
            outh = (
                pslot(i)[:, e0 * FD : (e0 + n) * FD]
                .rearrange("p (a c) -> p a c", a=n)[:, :, c0 : c0 + 512]
                .rearrange("p a (b d) -> p a b d", b=BC // 2)
            )
            return nc.vector.tensor_tensor(outh, in0h, wh, OP.mult)

        @block.vector
        def _(ve):
            w2seen = w3seen = w4seen = w5seen = False
            for i in range(RC):
                if i == 0:
                    if mask[0] and NSPLIT > 0:
                        ve.wait_ge(xsem, 1)
                        ve.wait_ge(swsem, 16)
                        w2seen = True
                        halfop(0, 0, 2, 0, XPC).then_inc(vsem, 1)
                        ve.wait_ge(xsem, 2)
                        halfop(0, 0, 2, 1, XPC).then_inc(vsem, 1)
                    else:
                        ve.wait_ge(xsem, 2)
                if i >= PRB:
                    ve.wait_ge(zsem, i - PRB + 1)
                if i < NSPLIT:
                    if not w2seen:
                        ve.wait_ge(swsem, 16)
                        w2seen = True
                    if mask[i]:
                        if i > 0:
                            # [q|m0] <- (Xp,Xt)*(w1,w0)
                            nc.vector.tensor_tensor(
                                pview(i, 0, 2), dgrp(XPC, 2), wgrp(i, 0, 2),
                                OP.mult,
                            ).then_inc(vsem, 1)
                    else:
                        nc.vector.tensor_tensor(
                            r3(pslot(i)[:, FD : 2 * FD]), r3(Xt), wbc(i, 1),
                            OP.mult,
                        ).then_inc(vsem, 1)
                    if not w3seen:
                        ve.wait_ge(s3sem, 16)
                        w3seen = True
                    # [m2|m3] <- (Mt,PDt)*(w2,w3)
                    nc.vector.tensor_tensor(
                        pview(i, 2, 2), dgrp(MC, 2), wgrp(i, 2, 2), OP.mult
                    ).then_inc(vsem, 1)
                else:
                    if not w4seen:
                        ve.wait_ge(s4sem, 16)
                        w4seen = True
                    if i >= 6 and not w5seen:
                        ve.wait_ge(s5sem, 16)
                        w5seen = True
                    if mask[i]:
                        if i == RC - 1:
                            rr = i - (WLAG - 1)
                            ve.wait_ge(lsem, rr + 1)
                            if rr >= LB:
                                ve.wait_ge(msem, rr - LB + 1)
                            nc.vector.tensor_tensor(
                                r3(wlslot(rr)), r3(latslot(rr)), wbc(rr, 4),
                                OP.mult,
                            ).then_inc(vsem, 1)
                            # halved quad: h0 then h1 so the PE's last
                            # z-group h0 overlaps the h1 product
                            for h in (0, 1):
                                c0 = h * 512
                                in0h = (
                                    BIG[:, 0 : 4 * STRD]
                                    .rearrange("p (a c) -> p a c", a=4)[
                                        :, :, c0 : c0 + 512
                                    ]
                                    .rearrange(
                                        "p a (b d) -> p a b d", b=BC // 2
                                    )
                                )
                                wh = (
                                    BIG[:, wcol(i) : wcol(i) + 4 * D]
                                    .rearrange("p (a d) -> p a d", a=4)
                                    .unsqueeze(2)
                                    .broadcast_to([L, 4, BC // 2, D])
                                )
                                outh = (
                                    pslot(i)
                                    .rearrange("p (a c) -> p a c", a=4)[
                                        :, :, c0 : c0 + 512
                                    ]
                                    .rearrange(
                                        "p a (b d) -> p a b d", b=BC // 2
                                    )
                                )
                                nc.vector.tensor_tensor(
                                    outh, in0h, wh, OP.mult
                                ).then_inc(vsem, 1)
                        else:
                            # [q|m0|m2|m3] <- (Xp,Xt,Mt,PDt)*(w1,w0,w2,w3)
                            nc.vector.tensor_tensor(
                                pview(i, 0, 4), dgrp(XPC, 4), wgrp(i, 0, 4),
                                OP.mult,
                            ).then_inc(vsem, 1)
                    else:
                        # [m0|m2|m3] <- (Xt,Mt,PDt)*(w0,w2,w3)
                        nc.vector.tensor_tensor(
                            pview(i, 1, 3), dgrp(XC, 3), wgrp(i, 1, 3), OP.mult
                        ).then_inc(vsem, 1)
                if not mask[i]:
                    ve.wait_ge(asem, kidx[i] + 1)
                    nc.vector.tensor_mul(
                        gbuf[:, :], Xp[:, :], kslot(kidx[i])
                    ).then_inc(vsem, 1)
                    nc.vector.tensor_tensor(
                        r3(pslot(i)[:, 0:FD]), r3(gbuf[:, :]), wbc(i, 0),
                        OP.mult,
                    ).then_inc(vsem, 1)
                if not WLPOOL and i >= WLAG - 1 and not (
                    i == RC - 1 and mask[i] and i >= NSPLIT
                ):
                    rr = i - (WLAG - 1)
                    if rr <= RC - NTAIL - 1:
                        ve.wait_ge(lsem, rr + 1)
                        if rr >= LB:
                            ve.wait_ge(msem, rr - LB + 1)
                        nc.vector.tensor_tensor(
                            r3(wlslot(rr)), r3(latslot(rr)), wbc(rr, 4),
                            OP.mult,
                        ).then_inc(vsem, 1)
            if not WLPOOL:
                for rr in range(RC - (WLAG - 1), RC - NTAIL):
                    ve.wait_ge(lsem, rr + 1)
                    ve.wait_ge(msem, rr - LB + 1)
                    nc.vector.tensor_tensor(
                        r3(wlslot(rr)), r3(latslot(rr)), wbc(rr, 4), OP.mult
                    ).then_inc(vsem, 1)
            # fused relu*wv straight from PSUM for the last iteration(s);
            # the very last one in halves so the out matmuls/relus pipeline
            if NTAIL >= 2:
                rr = RC - 2
                ve.wait_ge(zsem, rr + 1)
                ve.wait_ge(msem, rr - LB + 1)
                nc.vector.scalar_tensor_tensor(
                    r3(wlslot(rr)), r3(psz[rr % ZB][:, :]), 0.0, wbc(rr, 4),
                    OP.max, OP.mult,
                ).then_inc(vsem, 1)
            rr = RC - 1
            ve.wait_ge(msem, rr - LB + 1)
            for h, zwait in ((0, 1), (1, 2)):
                ve.wait_ge(z2sem, zwait)
                c0, c1 = h * 512, (h + 1) * 512
                wl3 = wlslot(rr)[:, c0:c1].rearrange("p (b d) -> p b d", b=BC // 2)
                pz3 = psz[rr % ZB][:, c0:c1].rearrange(
                    "p (b d) -> p b d", b=BC // 2
                )
                nc.vector.scalar_tensor_tensor(
                    wl3, pz3, 0.0, wbch(rr, 4), OP.max, OP.mult
                ).then_inc(vsem, 1)

        @block.gpsimd
        def _(gp):
            nc.gpsimd.memset(wsc[:, :], 1.0).then_inc(gsem, 1)
            gp.dma_start(out=OBA[:, :], in_=oba_e[:, :]).then_inc(obasem, 16)
            gp.wait_ge(obasem, 16)
            gp.dma_start(out=cft[:, :], in_=cft_e[:, :]).then_inc(cftsem, 16)
            gp.dma_start(out=OBB[:, :], in_=obb_e[:, :]).then_inc(obbsem, 16)
            for rr in range(RC - NTAIL if WLPOOL else 0):
                gp.wait_ge(lsem, rr + 1)
                if rr >= LB:
                    gp.wait_ge(msem, rr - LB + 1)
                nc.gpsimd.tensor_tensor(
                    r3(wlslot(rr)), r3(latslot(rr)), wbc(rr, 4), OP.mult
                ).then_inc(psem, 1)

        @block.tensor
        def _(te):
            # warmup: keep the PE out of its low p-state until real work
            # arrives. Results never read; ps0 reset by the real start=True.
            te.wait_ge(gsem, 1)
            for _w in range(NWARM):
                nc.tensor.matmul(
                    ps0[:, :], wsc[:, 0:RC], wsc[:, :],
                    start=True, stop=True, skip_group_check=True,
                )
            te.wait_ge(cstsem, 16)
            for i in range(RC):
                last = i == RC - 1
                if i >= ZB and i - ZB <= RC - NTAIL - 1:
                    te.wait_ge(lsem, i - ZB + 1)
                pz = psz[i % ZB]
                # (wait_key, first_prod_slot, nprods) groups in psum order
                if i == 0 and mask[0] and NSPLIT > 0:
                    groups = [(("p1h0", 0), "h0", 2), (("p1h1", 0), "h1", 2),
                              (("p2", 0), 2, 2)]
                elif i < NSPLIT:
                    if mask[i]:
                        groups = [(("p1", i), 0, 2), (("p2", i), 2, 2)]
                    else:
                        groups = [(("m0s", i), 1, 1), (("p2", i), 2, 2),
                                  (("qs", i), 0, 1)]
                elif mask[i]:
                    groups = [(("quad", i), 0, 4)]
                else:
                    groups = [(("tri", i), 1, 3), (("qs", i), 0, 1)]
                if last:
                    # h0 stream first, then h1, each closed separately so the
                    # DVE's stt halves overlap with this group's tail
                    fast_last = mask[i] and i >= NSPLIT
                    for h in (0, 1):
                        first = True
                        if fast_last:
                            te.wait_ge(vsem, C[(f"qh{h}", i)])
                        for key, e0, n in groups:
                            if h == 0 and not fast_last:
                                te.wait_ge(vsem, C[key])
                            for j in range(e0, e0 + n):
                                c0 = j * FD + h * 512
                                nc.tensor.matmul(
                                    pz[:, h * 512 : (h + 1) * 512], Ident,
                                    pslot(i)[:, c0 : c0 + 512],
                                    start=first, stop=False,
                                    skip_group_check=True,
                                )
                                first = False
                        nc.tensor.matmul(
                            pz[:, h * 512 : (h + 1) * 512], btsl(i), OHD,
                            start=False, stop=True, skip_group_check=True,
                        ).then_inc(z2sem, 1)
                else:
                    first_h = [True, True]
                    for key, e0, n in groups:
                        te.wait_ge(vsem, C[key])
                        if e0 == "h0" or e0 == "h1":
                            h = 0 if e0 == "h0" else 1
                            for j in range(n):
                                c0 = j * FD + h * 512
                                nc.tensor.matmul(
                                    pz[:, h * 512 : (h + 1) * 512], Ident,
                                    pslot(i)[:, c0 : c0 + 512],
                                    start=first_h[h], stop=False,
                                    skip_group_check=True,
                                )
                                first_h[h] = False
                            continue
                        for j in range(e0, e0 + n):
                            for h in (0, 1):
                                c0 = j * FD + h * 512
                                nc.tensor.matmul(
                                    pz[:, h * 512 : (h + 1) * 512], Ident,
                                    pslot(i)[:, c0 : c0 + 512],
                                    start=first_h[h], stop=False,
                                    skip_group_check=True,
                                )
                                first_h[h] = False
                    if i == 0:
                        te.wait_ge(obasem, 16)
                    elif i == 4:
                        te.wait_ge(obbsem, 16)
                    for h in (0, 1):
                        mm = nc.tensor.matmul(
                            pz[:, h * 512 : (h + 1) * 512], btsl(i), OHD,
                            start=False, stop=True, skip_group_check=True,
                        )
                        if h == 1:
                            mm.then_inc(zsem, 1)
                if i >= WLAG:
                    rr = i - WLAG
                    if WLPOOL:
                        te.wait_ge(psem, rr + 1)
                    else:
                        te.wait_ge(vsem, C[("wl", rr)])
                    wl = wlslot(rr)
                    nc.tensor.matmul(
                        ps0[:, :], OH3[:, rr, :], wl[:, 0:512],
                        start=(rr == 0), stop=False, skip_group_check=True,
                    )
                    nc.tensor.matmul(
                        ps1[:, :], OH3[:, rr, :], wl[:, 512:1024],
                        start=(rr == 0), stop=False, skip_group_check=True,
                    ).then_inc(msem, 1)
                    if rr == 0:
                        # accumulate the L*b_v rows early (order irrelevant)
                        nc.tensor.matmul(
                            ps0[:, :], lbvT, OHD,
                            start=False, stop=False, skip_group_check=True,
                        )
                        nc.tensor.matmul(
                            ps1[:, :], lbvT, OHD,
                            start=False, stop=False, skip_group_check=True,
                        )
            for rr in range(RC - WLAG, RC - NTAIL):
                if WLPOOL:
                    te.wait_ge(psem, rr + 1)
                else:
                    te.wait_ge(vsem, C[("wl", rr)])
                wl = wlslot(rr)
                nc.tensor.matmul(
                    ps0[:, :], OH3[:, rr, :], wl[:, 0:512],
                    start=False, stop=False, skip_group_check=True,
                )
                nc.tensor.matmul(
                    ps1[:, :], OH3[:, rr, :], wl[:, 512:1024],
                    start=False, stop=False, skip_group_check=True,
                ).then_inc(msem, 1)
            if NTAIL >= 2:
                rr = RC - 2
                te.wait_ge(vsem, C[("stt", rr)])
                wl = wlslot(rr)
                nc.tensor.matmul(
                    ps0[:, :], OH3[:, rr, :], wl[:, 0:512],
                    start=False, stop=False, skip_group_check=True,
                )
                nc.tensor.matmul(
                    ps1[:, :], OH3[:, rr, :], wl[:, 512:1024],
                    start=False, stop=False, skip_group_check=True,
                ).then_inc(msem, 1)
            rr = RC - 1
            wl = wlslot(rr)
            te.wait_ge(vsem, C[("stta", rr)])
            nc.tensor.matmul(
                ps0[:, :], OH3[:, rr, :], wl[:, 0:512],
                start=False, stop=True, skip_group_check=True,
            ).then_inc(msem, 1)
            te.wait_ge(vsem, C[("sttb", rr)])
            nc.tensor.matmul(
                ps1[:, :], OH3[:, rr, :], wl[:, 512:1024],
                start=False, stop=True, skip_group_check=True,
            ).then_inc(msem, 1)

    return nc


_CACHE = {}


def _buckets(a):
    """Deal r-indices into NK buckets of RC; zeros occupy each bucket's
    fast-mask positions first. Returns (buckets, nfast)."""
    zeros = [r for r in range(R) if a[r] == 0.0]
    pos = [r for r in range(R) if a[r] != 0.0]
    zbuck = [[] for _ in range(NK)]
    for j, r in enumerate(zeros):
        zbuck[j % NK].append(r)
    nfast = min(min(len(zb) for zb in zbuck), RC)
    mask = _fast_mask(nfast)
    pi = 0
    buckets = []
    for k in range(NK):
        zq = list(zbuck[k])
        rl = [None] * RC
        for i in range(RC):
            if mask[i]:
                rl[i] = zq.pop(0)
        for i in range(RC):
            if rl[i] is None:
                if zq:
                    rl[i] = zq.pop(0)
                else:
                    rl[i] = pos[pi]
                    pi += 1
        buckets.append(rl)
    return buckets, nfast


def _prepare(X, T, M, PD, alpha, w_v, w_t, b_t, b_v, ref_time):
    """Pack full inputs into per-core DRAM parameter maps.
    Returns (mask, buckets, in_maps)."""
    a = np.maximum(alpha.reshape(R), 0.0)
    s_ref = ref_time.reshape(R)
    nas = -(a * s_ref)
    bt4 = 4.0 * b_t[..., 0]              # [R, L, D]
    lbv = float(L) * b_v[:, 0, :]        # [R, D]

    buckets, nfast = _buckets(a)
    mask = _fast_mask(nfast)

    # per-r params: [w1|w0|w2|w3|wv] (5*D per iter)
    wts = np.stack(
        [w_t[..., 1], w_t[..., 0], w_t[..., 2], w_t[..., 3], w_v], axis=2
    )                                     # [R, L, 5, D]

    oh = np.zeros((L, RC, RC), np.float32)
    for r in range(RC):
        oh[:, r, r] = 1.0
    ident = np.eye(L, dtype=np.float32)
    ohd = np.zeros((64, 512), np.float32)
    for b in range(8):
        for d in range(64):
            ohd[d, b * 64 + d] = 1.0

    cstf = np.zeros((L, CONST_W), np.float32)
    cstf[:, 0 : RC * RC] = oh.reshape(L, RC * RC)
    cstf[:, RC * RC :] = ident
    cst = cstf.astype(_nbf16)

    in_maps = []
    for c in range(8):
        b0 = (c // NK) * BC
        rl = buckets[c % NK]
        tr = lambda x: np.ascontiguousarray(
            x[b0 : b0 + BC].transpose(1, 0, 2).reshape(L, FD)
        )
        cfc = np.zeros((L, CFC_W), np.float32)
        cfc[:, 0:RC] = a[rl]
        cfc[:, RC : 2 * RC] = nas[rl]
        cbf = np.zeros((L, CB_W), np.float32)
        trx = tr(X)
        cbf[:, XH0E : XH0E + 512] = trx[:, 0:512]
        cbf[:, XH1E : XH1E + 512] = trx[:, 512:FD]
        cbf[:, MTE : MTE + FD] = tr(M)
        cbf[:, PDE : PDE + FD] = tr(PD)
        for i, r in enumerate(rl):
            if i < 2:
                base = W01E + i * WS_W
            elif i < 6:
                base = W25E + (i - 2) * WS_W
            else:
                base = W611E + (i - 6) * WS_W
            cbf[:, base : base + WS_W] = wts[r].reshape(L, WS_W)
        oba = np.zeros((64, OBA_W), np.float32)
        oba[:, 0:512] = ohd
        obb = np.zeros((64, OBB_W), np.float32)
        for i, r in enumerate(rl):
            oba[0:D, 512 + i] = lbv[r]
            if i < 4:
                oba[0:D, 512 + 16 + i * L : 512 + 16 + (i + 1) * L] = bt4[r].T
            else:
                obb[0:D, (i - 4) * L : (i - 3) * L] = bt4[r].T
        in_maps.append(
            {
                "cfc": cfc,
                "cfT": tr(T).astype(_nbf16),
                "cb": np.ascontiguousarray(cbf).astype(_nbf16),
                "CST": cst,
                "OBA": oba.astype(_nbf16),
                "OBB": obb.astype(_nbf16),
            }
        )
    return mask, buckets, in_maps


def kernel(X, T, M, PD, alpha, w_v, w_t, b_t, b_v, ref_time):
    X = np.asarray(X, np.float32)
    T = np.asarray(T, np.float32)
    M = np.asarray(M, np.float32)
    PD = np.asarray(PD, np.float32)
    alpha = np.asarray(alpha, np.float32)
    w_v = np.asarray(w_v, np.float32)
    w_t = np.asarray(w_t, np.float32)
    b_t = np.asarray(b_t, np.float32)
    b_v = np.asarray(b_v, np.float32)
    ref_time = np.asarray(ref_time, np.float32)

    mask, buckets, in_maps = _prepare(
        X, T, M, PD, alpha, w_v, w_t, b_t, b_v, ref_time
    )

    if mask not in _CACHE:
        _CACHE[mask] = _build_graph(mask)
    nc = _CACHE[mask]

    trace = bool(os.environ.get("BASS_KERNEL_TRACE"))
    kw = {}
    if trace:
        tmpdir = os.environ.get("BASS_KERNEL_TRACE_DIR") or None
        kw = dict(trace=True, tmpdir=tmpdir)
    for _attempt in range(3):
        res = run_bass_kernel_spmd(nc, in_maps, core_ids=list(range(8)), **kw)
        outs = [
            np.asarray(res.results[c]["out"], np.float32) for c in range(8)
        ]
        if all(np.isfinite(o).all() for o in outs):
            break
    if trace:
        _CACHE["exec_time_ns"] = res.exec_time_ns
        print(f"HW exec time: {res.exec_time_ns} ns")

    out = np.zeros((B, R, D), np.float32)
    for c in range(8):
        b0 = (c // NK) * BC
        rl = buckets[c % NK]
        o = outs[c].reshape(RC, BC, D)
        for i, r in enumerate(rl):
            out[b0 : b0 + BC, r] = o[i]
    return out


# revision 47
# speedup vs baseline: 1.0115x; 1.0115x over previous
"""ALNN layer kernel for 8 TRN2 NeuronCores (raw Bass, explicit semaphores).

out[b,r,d] = relu( sum_l w_v[r,l,d]*relu(z[b,r,l,d]) + L*b_v[r,d] )
z = wt0*X + wt1*relu(X)*k + wt2*M + wt3*PD + 4*bt
k = exp(-relu(alpha_r)*|T - s_r|)        (uses relu(X*k) == relu(X)*k, k>0)

Sharding: B split 2 ways x R dealt into 4 buckets -> 8 cores, 16 b x 12 r
each. Cores c and c+4 share bucket c%4. r's with relu(alpha)=0 take a
fast path (k == 1). The graph is SPMD-uniform: a shared fast-mask sized
by the min zero-count across buckets.

v8 (from v7 @ 60816ns -> ~56.5us measured): DVE-bound design.
 - DVE is the saturated engine (~40.5us): products merged into ONE
   quad op [q|m0|m2|m3] per fast iter (FD=4096 @2x_1p; tensor_tensor
   is capped at 2x — no 4x uop) or tri+g+qs (slow). wl = lat*wv also
   on DVE (gpsimd tensor ops are ~2x slower than modeled AND slow
   concurrent DVE ops ~4x via SBUF contention — do not use).
 - Slow slots interleaved at 3,5,7,9 (SLOW_FILL) so ACT-heavy abs/exp
   iters alternate with DVE-heavy slow-product iters; abs/exp pair j
   emitted at ACT iter slows[j]-1.
 - PE: z via identity matmuls (one vsem wait per DVE op), bt via
   btsl x OHD trick, out-reduce via one-hot; ~14 warmup matmuls hold
   the p-state until real work. Every psum column's first write needs
   start=True; reading a psum bank mid-accumulation-group faults HW.
 - ACT: relu + slow-iter abs/exp; a dummy activation right after the
   cfc wait forces the lazy ~1.3us ACT_TABLE_LOAD off the critical
   path. Final relus read ps0/ps1 only after msem 12/13 (groups
   closed); output as bf16, column-split DMAs from the sync ring.
 - DMA: ~7.3us fixed framework preamble, then ~230-260GB/s aggregate
   over 16 engines shared by all rings. Consumption-ordered chunks:
   sync ring [Xt-h0 | W0W1 | Xt-h1], deferred [W2-5 | W6-11] after
   s3; ACT ring cfc, CST, Mt, PDt; gpsimd SWDGE ring OBA, then
   (deferred) cfT, OBB. Same-ring DMA completion is FIFO per engine,
   so a later chunk's semaphore implies earlier chunks landed.
 - Tail: last iter's quad split into column halves feeding the PE's
   half-closed (z2sem) last z-group; wl via scalar_tensor_tensor
   straight from PSUM; NaN-retry guard for a rare (~1/20) transient.

Raw bass: at most ONE attached sync-wait per compute instruction, so
cross-engine deps use standalone wait_ge; each DMA gets a dedicated
semaphore (two DMAs sharing one sem can interleave per-queue completions,
so a partial wait would be unsound).
"""

import os
import numpy as np
import ml_dtypes

import concourse.bass as bass
import concourse.mybir as mybir
from concourse.bass_utils import run_bass_kernel_spmd

AF = mybir.ActivationFunctionType
OP = mybir.AluOpType
BF16 = mybir.dt.bfloat16
F32 = mybir.dt.float32

B, R, L, D = 32, 48, 128, 64
NB, NK = 2, 4              # b-halves x r-buckets = 8 cores
BC, RC = B // NB, R // NK  # 16 b, 12 r per core
FD = BC * D                # 1024 free elems

CFC_W = 2 * RC             # f32 consts: [Abc 12 | NASbc 12]
WS_W = 5 * D               # per-iter param slice: [w1|w0|w2|w3|wv]
STRD = FD + 2 * WS_W       # 1664: uniform stride of the data planes

# BIG sbuf layout (bf16), stride-1664 planes:
XPC = 0                    # Xp   [0:1024]
WSCC = 1024                # wsc  [1024:1536] (warmup ones), pad to 1664
XC = STRD                  # Xt   [1664:2688] | pad [2688:3328]
MC = 2 * STRD              # Mt   [3328:4352] | pad
PDC = 3 * STRD             # PDt  [4992:6016]
W0C = PDC + FD             # W0..W11 [6016:9856]
GC = W0C + 12 * WS_W       # g    [9856:10880]
BIG_W = GC + FD            # 10880

# DRAM cb stream, consumption order:
# [Xt-h0 | W0 W1 | Xt-h1 | Mt | PDt | W2-5 | W6-11]
CB_W = 6912
XH0E = 0
W01E = 512
XH1E = W01E + 2 * WS_W     # 1152
MTE = XH1E + 512           # 1664
PDE = MTE + FD             # 2688
W25E = PDE + FD            # 3712
W611E = W25E + 4 * WS_W    # 4992

CONST_W = RC * RC + L      # [OH3 144 | Ident 128]
OBA_W = 512 + 16 + 4 * L   # [OHD | lbvT(16 pad) | BT0-3]
OBB_W = 8 * L              # [BT4-11]


def wcol(i):
    return W0C + i * WS_W

_nbf16 = ml_dtypes.bfloat16

WLPOOL = False  # gpsimd tensor ops are slow and poison concurrent DVE ops

LB = 5    # wl buffers (Pool/DVE -> PE)
LAB = 4   # lat buffers (ACT -> Pool)
PRB = 3   # product buffer sets (DVE -> PE)
ZB = 3    # psum z triple-buffer (6 of 8 banks; ps0/ps1 take the rest)
WLAG = 3  # out-mm for rr = i - WLAG emitted in PE iter i
ALAG = 2  # relu for rr = i - ALAG emitted in ACT iter i
NWARM = 14  # PE warmup matmuls (keep PE out of low p-state until work)
NSPLIT = 2  # iters with split product ops (partial-input start)
NTAIL = 1   # last iters: wl via DVE stt straight from PSUM


SLOW_FILL = (3, 5, 7, 9, 2, 4, 6, 8, 1, 10, 0, 11)


def _fast_mask(nfast):
    """Interleave slow slots (odd-first) so ACT-heavy abs/exp iterations
    alternate with DVE-heavy slow-product iterations; head and tail slots
    stay fast when possible (early start, short drain)."""
    nslow = RC - nfast
    mask = [True] * RC
    for s in SLOW_FILL[:nslow]:
        mask[s] = False
    return tuple(mask)


def _dve_schedule(mask):
    """DVE emission order; must match the @block.vector body exactly."""
    sched = []
    for i in range(RC):
        if i == 0 and mask[0] and NSPLIT > 0:
            sched.append(("p1h0", 0))
            sched.append(("p1h1", 0))
        if i < NSPLIT:
            if mask[i]:
                if i > 0:
                    sched.append(("p1", i))    # [q|m0]
            else:
                sched.append(("m0s", i))
            sched.append(("p2", i))        # [m2|m3]
            if not mask[i]:
                sched.append(("g", i))
                sched.append(("qs", i))
        else:
            if mask[i]:
                if i == RC - 1:
                    if RC - 1 >= WLAG - 1:
                        sched.append(("wl", RC - 1 - (WLAG - 1)))
                    sched.append(("qh0", i))
                    sched.append(("qh1", i))
                else:
                    sched.append(("quad", i))  # [q|m0|m2|m3]
            else:
                sched.append(("tri", i))   # [m0|m2|m3]
                sched.append(("g", i))
                sched.append(("qs", i))
        if not WLPOOL and i >= WLAG - 1 and not (
            i == RC - 1 and mask[i] and i >= NSPLIT
        ):
            rr = i - (WLAG - 1)
            if rr <= RC - NTAIL - 1:
                sched.append(("wl", rr))
    if not WLPOOL:
        for rr in range(RC - (WLAG - 1), RC - NTAIL):
            sched.append(("wl", rr))
    if NTAIL >= 2:
        sched.append(("stt", RC - 2))
    sched.append(("stta", RC - 1))
    sched.append(("sttb", RC - 1))
    return sched


def _build_graph(mask, detect_races=True):
    nslow = sum(1 for f in mask if not f)
    ksl = max(nslow, 1)
    slows = [i for i in range(RC) if not mask[i]]
    kidx = {i: j for j, i in enumerate(slows)}  # slow iter -> k slot
    sched = _dve_schedule(mask)
    C = {key: idx + 1 for idx, key in enumerate(sched)}

    nc = bass.Bass(detect_race_conditions=detect_races)
    cfc_e = nc.declare_dram_parameter("cfc", [L, CFC_W], F32, isOutput=False)
    cft_e = nc.declare_dram_parameter("cfT", [L, FD], BF16, isOutput=False)
    cb_e = nc.declare_dram_parameter("cb", [L, CB_W], BF16, isOutput=False)
    cst_e = nc.declare_dram_parameter("CST", [L, CONST_W], BF16, isOutput=False)
    oba_e = nc.declare_dram_parameter("OBA", [64, OBA_W], BF16, isOutput=False)
    obb_e = nc.declare_dram_parameter("OBB", [64, OBB_W], BF16, isOutput=False)
    out_e = nc.declare_dram_parameter("out", [RC, FD], BF16, isOutput=True)

    from contextlib import ExitStack

    with ExitStack() as ctx:
        e = ctx.enter_context
        cfc = e(nc.sbuf_tensor([L, CFC_W], F32))
        cft = e(nc.sbuf_tensor([L, FD], BF16))
        BIG = e(nc.sbuf_tensor([L, BIG_W], BF16))
        CST = e(nc.sbuf_tensor([L, CONST_W], BF16))
        OBA = e(nc.sbuf_tensor([64, OBA_W], BF16))
        OBB = e(nc.sbuf_tensor([64, OBB_W], BF16))
        dist = e(nc.sbuf_tensor([L, FD], F32))
        kbuf = e(nc.sbuf_tensor([L, ksl * FD], BF16))
        prods = e(nc.sbuf_tensor([L, PRB * 4 * FD], BF16))
        latb = e(nc.sbuf_tensor([L, LAB * FD], BF16))
        wlbuf = e(nc.sbuf_tensor([L, LB * FD], BF16))
        outt = e(nc.sbuf_tensor([RC, FD], BF16))
        psz = [e(nc.psum_tensor(f"psz{j}", [L, FD], F32)) for j in range(ZB)]
        ps0 = e(nc.psum_tensor([RC, 512], F32))
        ps1 = e(nc.psum_tensor([RC, 512], F32))
        s1sem = e(nc.semaphore("s1sem"))    # Xt-h0
        s1bsem = e(nc.semaphore("s1bsem"))  # Xt-h1
        swsem = e(nc.semaphore("swsem"))    # W0|W1
        s2sem = e(nc.semaphore("s2sem"))    # cb c2: Mt|W2|W3
        s3sem = e(nc.semaphore("s3sem"))    # cb c3: PDt|W4|W5
        s4sem = e(nc.semaphore("s4sem"))    # cb c4: W2-5
        s5sem = e(nc.semaphore("s5sem"))    # cb c5: W6-11
        cfcsem = e(nc.semaphore("cfcsem"))
        cstsem = e(nc.semaphore("cstsem"))  # OH3|Ident
        cftsem = e(nc.semaphore("cftsem"))
        obasem = e(nc.semaphore("obasem"))  # OHD|lbvT|BT0-3
        obbsem = e(nc.semaphore("obbsem"))  # BT4-11
        asem = e(nc.semaphore("asem"))      # ACT k completions
        lsem = e(nc.semaphore("lsem"))      # ACT lat relu completions
        psem = e(nc.semaphore("psem"))      # Pool wl completions
        zsem = e(nc.semaphore("zsem"))      # PE z-group completions (1/iter)
        z2sem = e(nc.semaphore("z2sem"))    # PE last z-group half completions
        msem = e(nc.semaphore("msem"))      # PE out-mm completions
        vsem = e(nc.semaphore("vsem"))      # DVE op completions
        osem = e(nc.semaphore("osem"))
        rsem = e(nc.semaphore("rsem"))
        gsem = e(nc.semaphore("gsem"))
        xsem = e(nc.semaphore("xsem"))  # ACT-computed Xp halves
        block = e(nc.Block())

        Abc = cfc[:, 0:RC]
        NASbc = cfc[:, RC : 2 * RC]
        Xp = BIG[:, XPC : XPC + FD]
        Xt = BIG[:, XC : XC + FD]
        Mt = BIG[:, MC : MC + FD]
        PDt = BIG[:, PDC : PDC + FD]
        gbuf = BIG[:, GC : GC + FD]
        wsc = BIG[:, WSCC : WSCC + 512]
        OH3 = CST[:, 0 : RC * RC].rearrange("p (r m) -> p r m", r=RC)
        Ident = CST[:, RC * RC : RC * RC + L]
        OHD = OBA[:, 0:512]
        lbvT = OBA[:, 512 : 512 + RC]

        def r3(ap):
            return ap.rearrange("p (b d) -> p b d", b=BC)

        def kslot(j):
            return kbuf[:, j * FD : (j + 1) * FD]

        def wbc(i, ei):
            base = wcol(i) + ei * D
            return BIG[:, base : base + D].unsqueeze(1).broadcast_to([L, BC, D])

        def wbch(i, ei):
            base = wcol(i) + ei * D
            return (
                BIG[:, base : base + D].unsqueeze(1).broadcast_to([L, BC // 2, D])
            )

        def wgrp(i, e0, n):
            """weights slots e0..e0+n-1 of iter i: [L, n, 16, 64] bcast."""
            base = wcol(i) + e0 * D
            return (
                BIG[:, base : base + n * D]
                .rearrange("p (a d) -> p a d", a=n)
                .unsqueeze(2)
                .broadcast_to([L, n, BC, D])
            )

        def dgrp(c0, n):
            """n data planes at stride STRD from col c0: [L, n, 16, 64]."""
            return (
                BIG[:, c0 : c0 + n * STRD]
                .rearrange("p (a c) -> p a c", a=n)[:, :, 0:FD]
                .rearrange("p a (b d) -> p a b d", b=BC)
            )

        def pslot(i):
            return prods[:, (i % PRB) * 4 * FD : (i % PRB + 1) * 4 * FD]

        def pview(i, e0, n):
            s = pslot(i)
            return s[:, e0 * FD : (e0 + n) * FD].rearrange(
                "p (a b d) -> p a b d", a=n, b=BC
            )

        def btsl(i):
            if i < 4:
                return OBA[:, 512 + 16 + i * L : 512 + 16 + (i + 1) * L]
            return OBB[:, (i - 4) * L : (i - 3) * L]

        def latslot(rr):
            return latb[:, (rr % LAB) * FD : (rr % LAB + 1) * FD]

        def wlslot(rr):
            return wlbuf[:, (rr % LB) * FD : (rr % LB + 1) * FD]

        @block.sync
        def _(sp):
            sp.dma_start(
                out=BIG[:, XC : XC + 512], in_=cb_e[:, XH0E : XH0E + 512]
            ).then_inc(s1sem, 16)
            sp.dma_start(
                out=BIG[:, W0C : W0C + 2 * WS_W],
                in_=cb_e[:, W01E : W01E + 2 * WS_W],
            ).then_inc(swsem, 16)
            sp.dma_start(
                out=BIG[:, XC + 512 : XC + FD], in_=cb_e[:, XH1E : XH1E + 512]
            ).then_inc(s1bsem, 16)
            sp.wait_ge(s3sem, 16)
            sp.dma_start(
                out=BIG[:, W0C + 2 * WS_W : W0C + 6 * WS_W],
                in_=cb_e[:, W25E : W25E + 4 * WS_W],
            ).then_inc(s4sem, 16)
            sp.dma_start(
                out=BIG[:, W0C + 6 * WS_W : GC], in_=cb_e[:, W611E:]
            ).then_inc(s5sem, 16)
            sp.wait_ge(rsem, 1)
            sp.dma_start(out=out_e[:, 0:512], in_=outt[:, 0:512]).then_inc(
                osem, 16
            )
            sp.wait_ge(rsem, 2)
            sp.dma_start(out=out_e[:, 512:], in_=outt[:, 512:]).then_inc(
                osem, 16
            )

        @block.scalar
        def _(act):
            act.dma_start(out=cfc[:, :], in_=cfc_e[:, :]).then_inc(cfcsem, 16)
            act.dma_start(out=CST[:, :], in_=cst_e[:, :]).then_inc(cstsem, 16)
            act.dma_start(
                out=BIG[:, MC : MC + FD], in_=cb_e[:, MTE : MTE + FD]
            ).then_inc(s2sem, 16)
            act.dma_start(
                out=BIG[:, PDC : PDC + FD], in_=cb_e[:, PDE : PDE + FD]
            ).then_inc(s3sem, 16)
            act.wait_ge(cfcsem, 16)
            # dummy op: forces the lazy ACT_TABLE_LOAD (~1.3us) to happen
            # now, while ACT is idle, instead of before the first relu
            nc.scalar.activation(dist[0:1, 0:1], cfc[0:1, 0:1], AF.Relu)
            # Xp = relu(Xt) on the idle ACT engine (frees ~0.6us of DVE)
            act.wait_ge(s1sem, 16)
            nc.scalar.activation(Xp[:, 0:512], Xt[:, 0:512], AF.Relu).then_inc(
                xsem, 1
            )
            act.wait_ge(s1bsem, 16)
            nc.scalar.activation(Xp[:, 512:FD], Xt[:, 512:FD], AF.Relu).then_inc(
                xsem, 1
            )
            pj = [max(1, s - 1) for s in slows]
            cft_waited = False
            for i in range(RC):
                if ALAG <= i and i - ALAG <= RC - NTAIL - 1:
                    rr = i - ALAG
                    act.wait_ge(zsem, rr + 1)
                    if rr >= LAB:
                        if WLPOOL:
                            act.wait_ge(psem, rr - LAB + 1)
                        else:
                            act.wait_ge(vsem, C[("wl", rr - LAB)])
                    nc.scalar.activation(
                        latslot(rr), psz[rr % ZB][:, :], AF.Relu
                    ).then_inc(lsem, 1)
                for j in range(nslow):
                    if pj[j] != i:
                        continue
                    if not cft_waited:
                        act.wait_ge(cftsem, 16)
                        cft_waited = True
                    si = slows[j]
                    nc.scalar.activation(
                        dist[:, :], cft[:, :], AF.Abs,
                        bias=NASbc[:, si : si + 1], scale=Abc[:, si : si + 1],
                    )
                    nc.scalar.activation(
                        kslot(j), dist[:, :], AF.Exp, scale=-1.0
                    ).then_inc(asem, 1)
            # drain the output in two row chunks: rows rr are final right
            # after iteration rr's out-mm (one-hot row writes)
            for rr in range(RC - ALAG, RC - NTAIL):
                act.wait_ge(zsem, rr + 1)
                act.wait_ge(vsem, C[("wl", rr - LAB)])
                nc.scalar.activation(
                    latslot(rr), psz[rr % ZB][:, :], AF.Relu
                ).then_inc(lsem, 1)
            act.wait_ge(msem, 12)
            nc.scalar.activation(outt[:, 0:512], ps0[:, :], AF.Relu).then_inc(
                rsem, 1
            )
            act.wait_ge(msem, 13)
            nc.scalar.activation(outt[:, 512:], ps1[:, :], AF.Relu).then_inc(
                rsem, 1
            )

        def halfop(i, e0, n, h, in0c):
            c0 = h * 512
            in0h = (
                BIG[:, in0c : in0c + n * STRD]
                .rearrange("p (a c) -> p a c", a=n)[:, :, c0 : c0 + 512]
                .rearrange("p a (b d) -> p a b d", b=BC // 2)
            )
            wh = (
                BIG[:, wcol(i) + e0 * D : wcol(i) + (e0 + n) * D]
                .rearrange("p (a d) -> p a d", a=n)
                .unsqueeze(2)
                .broadcast_to([L, n, BC // 2, D])
            )
            outh = (
                pslot(i)[:, e0 * FD : (e0 + n) * FD]
                .rearrange("p (a c) -> p a c", a=n)[:, :, c0 : c0 + 512]
                .rearrange("p a (b d) -> p a b d", b=BC // 2)
            )
            return nc.vector.tensor_tensor(outh, in0h, wh, OP.mult)

        @block.vector
        def _(ve):
            w2seen = w3seen = w4seen = w5seen = False
            for i in range(RC):
                if i == 0:
                    if mask[0] and NSPLIT > 0:
                        ve.wait_ge(xsem, 1)
                        ve.wait_ge(swsem, 16)
                        w2seen = True
                        halfop(0, 0, 2, 0, XPC).then_inc(vsem, 1)
                        ve.wait_ge(xsem, 2)
                        halfop(0, 0, 2, 1, XPC).then_inc(vsem, 1)
                    else:
                        ve.wait_ge(xsem, 2)
                if i >= PRB:
                    ve.wait_ge(zsem, i - PRB + 1)
                if i < NSPLIT:
                    if not w2seen:
                        ve.wait_ge(swsem, 16)
                        w2seen = True
                    if mask[i]:
                        if i > 0:
                            # [q|m0] <- (Xp,Xt)*(w1,w0)
                            nc.vector.tensor_tensor(
                                pview(i, 0, 2), dgrp(XPC, 2), wgrp(i, 0, 2),
                                OP.mult,
                            ).then_inc(vsem, 1)
                    else:
                        nc.vector.tensor_tensor(
                            r3(pslot(i)[:, FD : 2 * FD]), r3(Xt), wbc(i, 1),
                            OP.mult,
                        ).then_inc(vsem, 1)
                    if not w3seen:
                        ve.wait_ge(s3sem, 16)
                        w3seen = True
                    # [m2|m3] <- (Mt,PDt)*(w2,w3)
                    nc.vector.tensor_tensor(
                        pview(i, 2, 2), dgrp(MC, 2), wgrp(i, 2, 2), OP.mult
                    ).then_inc(vsem, 1)
                else:
                    if not w4seen:
                        ve.wait_ge(s4sem, 16)
                        w4seen = True
                    if i >= 6 and not w5seen:
                        ve.wait_ge(s5sem, 16)
                        w5seen = True
                    if mask[i]:
                        if i == RC - 1:
                            rr = i - (WLAG - 1)
                            ve.wait_ge(lsem, rr + 1)
                            if rr >= LB:
                                ve.wait_ge(msem, rr - LB + 1)
                            nc.vector.tensor_tensor(
                                r3(wlslot(rr)), r3(latslot(rr)), wbc(rr, 4),
                                OP.mult,
                            ).then_inc(vsem, 1)
                            # halved quad: h0 then h1 so the PE's last
                            # z-group h0 overlaps the h1 product
                            for h in (0, 1):
                                c0 = h * 512
                                in0h = (
                                    BIG[:, 0 : 4 * STRD]
                                    .rearrange("p (a c) -> p a c", a=4)[
                                        :, :, c0 : c0 + 512
                                    ]
                                    .rearrange(
                                        "p a (b d) -> p a b d", b=BC // 2
                                    )
                                )
                                wh = (
                                    BIG[:, wcol(i) : wcol(i) + 4 * D]
                                    .rearrange("p (a d) -> p a d", a=4)
                                    .unsqueeze(2)
                                    .broadcast_to([L, 4, BC // 2, D])
                                )
                                outh = (
                                    pslot(i)
                                    .rearrange("p (a c) -> p a c", a=4)[
                                        :, :, c0 : c0 + 512
                                    ]
                                    .rearrange(
                                        "p a (b d) -> p a b d", b=BC // 2
                                    )
                                )
                                nc.vector.tensor_tensor(
                                    outh, in0h, wh, OP.mult
                                ).then_inc(vsem, 1)
                        else:
                            # [q|m0|m2|m3] <- (Xp,Xt,Mt,PDt)*(w1,w0,w2,w3)
                            nc.vector.tensor_tensor(
                                pview(i, 0, 4), dgrp(XPC, 4), wgrp(i, 0, 4),
                                OP.mult,
                            ).then_inc(vsem, 1)
                    else:
                        # [m0|m2|m3] <- (Xt,Mt,PDt)*(w0,w2,w3)
                        nc.vector.tensor_tensor(
                            pview(i, 1, 3), dgrp(XC, 3), wgrp(i, 1, 3), OP.mult
                        ).then_inc(vsem, 1)
                if not mask[i]:
                    ve.wait_ge(asem, kidx[i] + 1)
                    nc.vector.tensor_mul(
                        gbuf[:, :], Xp[:, :], kslot(kidx[i])
                    ).then_inc(vsem, 1)
                    nc.vector.tensor_tensor(
                        r3(pslot(i)[:, 0:FD]), r3(gbuf[:, :]), wbc(i, 0),
                        OP.mult,
                    ).then_inc(vsem, 1)
                if not WLPOOL and i >= WLAG - 1 and not (
                    i == RC - 1 and mask[i] and i >= NSPLIT
                ):
                    rr = i - (WLAG - 1)
                    if rr <= RC - NTAIL - 1:
                        ve.wait_ge(lsem, rr + 1)
                        if rr >= LB:
                            ve.wait_ge(msem, rr - LB + 1)
                        nc.vector.tensor_tensor(
                            r3(wlslot(rr)), r3(latslot(rr)), wbc(rr, 4),
                            OP.mult,
                        ).then_inc(vsem, 1)
            if not WLPOOL:
                for rr in range(RC - (WLAG - 1), RC - NTAIL):
                    ve.wait_ge(lsem, rr + 1)
                    ve.wait_ge(msem, rr - LB + 1)
                    nc.vector.tensor_tensor(
                        r3(wlslot(rr)), r3(latslot(rr)), wbc(rr, 4), OP.mult
                    ).then_inc(vsem, 1)
            # fused relu*wv straight from PSUM for the last iteration(s);
            # the very last one in halves so the out matmuls/relus pipeline
            if NTAIL >= 2:
                rr = RC - 2
                ve.wait_ge(zsem, rr + 1)
                ve.wait_ge(msem, rr - LB + 1)
                nc.vector.scalar_tensor_tensor(
                    r3(wlslot(rr)), r3(psz[rr % ZB][:, :]), 0.0, wbc(rr, 4),
                    OP.max, OP.mult,
                ).then_inc(vsem, 1)
            rr = RC - 1
            ve.wait_ge(msem, rr - LB + 1)
            for h, zwait in ((0, 1), (1, 2)):
                ve.wait_ge(z2sem, zwait)
                c0, c1 = h * 512, (h + 1) * 512
                wl3 = wlslot(rr)[:, c0:c1].rearrange("p (b d) -> p b d", b=BC // 2)
                pz3 = psz[rr % ZB][:, c0:c1].rearrange(
                    "p (b d) -> p b d", b=BC // 2
                )
                nc.vector.scalar_tensor_tensor(
                    wl3, pz3, 0.0, wbch(rr, 4), OP.max, OP.mult
                ).then_inc(vsem, 1)

        @block.gpsimd
        def _(gp):
            nc.gpsimd.memset(wsc[:, :], 1.0).then_inc(gsem, 1)
            gp.dma_start(out=OBA[:, :], in_=oba_e[:, :]).then_inc(obasem, 16)
            gp.wait_ge(obasem, 16)
            gp.dma_start(out=cft[:, :], in_=cft_e[:, :]).then_inc(cftsem, 16)
            gp.dma_start(out=OBB[:, :], in_=obb_e[:, :]).then_inc(obbsem, 16)
            for rr in range(RC - NTAIL if WLPOOL else 0):
                gp.wait_ge(lsem, rr + 1)
                if rr >= LB:
                    gp.wait_ge(msem, rr - LB + 1)
                nc.gpsimd.tensor_tensor(
                    r3(wlslot(rr)), r3(latslot(rr)), wbc(rr, 4), OP.mult
                ).then_inc(psem, 1)

        @block.tensor
        def _(te):
            # warmup: keep the PE out of its low p-state until real work
            # arrives. Results never read; ps0 reset by the real start=True.
            te.wait_ge(gsem, 1)
            for _w in range(NWARM):
                nc.tensor.matmul(
                    ps0[:, :], wsc[:, 0:RC], wsc[:, :],
                    start=True, stop=True, skip_group_check=True,
                )
            te.wait_ge(cstsem, 16)
            for i in range(RC):
                last = i == RC - 1
                if i >= ZB and i - ZB <= RC - NTAIL - 1:
                    te.wait_ge(lsem, i - ZB + 1)
                pz = psz[i % ZB]
                # (wait_key, first_prod_slot, nprods) groups in psum order
                if i == 0 and mask[0] and NSPLIT > 0:
                    groups = [(("p1h0", 0), "h0", 2), (("p1h1", 0), "h1", 2),
                              (("p2", 0), 2, 2)]
                elif i < NSPLIT:
                    if mask[i]:
                        groups = [(("p1", i), 0, 2), (("p2", i), 2, 2)]
                    else:
                        groups = [(("m0s", i), 1, 1), (("p2", i), 2, 2),
                                  (("qs", i), 0, 1)]
                elif mask[i]:
                    groups = [(("quad", i), 0, 4)]
                else:
                    groups = [(("tri", i), 1, 3), (("qs", i), 0, 1)]
                if last:
                    # h0 stream first, then h1, each closed separately so the
                    # DVE's stt halves overlap with this group's tail
                    fast_last = mask[i] and i >= NSPLIT
                    for h in (0, 1):
                        first = True
                        if fast_last:
                            te.wait_ge(vsem, C[(f"qh{h}", i)])
                        for key, e0, n in groups:
                            if h == 0 and not fast_last:
                                te.wait_ge(vsem, C[key])
                            for j in range(e0, e0 + n):
                                c0 = j * FD + h * 512
                                nc.tensor.matmul(
                                    pz[:, h * 512 : (h + 1) * 512], Ident,
                                    pslot(i)[:, c0 : c0 + 512],
                                    start=first, stop=False,
                                    skip_group_check=True,
                                )
                                first = False
                        nc.tensor.matmul(
                            pz[:, h * 512 : (h + 1) * 512], btsl(i), OHD,
                            start=False, stop=True, skip_group_check=True,
                        ).then_inc(z2sem, 1)
                else:
                    first_h = [True, True]
                    for key, e0, n in groups:
                        te.wait_ge(vsem, C[key])
                        if e0 == "h0" or e0 == "h1":
                            h = 0 if e0 == "h0" else 1
                            for j in range(n):
                                c0 = j * FD + h * 512
                                nc.tensor.matmul(
                                    pz[:, h * 512 : (h + 1) * 512], Ident,
                                    pslot(i)[:, c0 : c0 + 512],
                                    start=first_h[h], stop=False,
                                    skip_group_check=True,
                                )
                                first_h[h] = False
                            continue
                        for j in range(e0, e0 + n):
                            for h in (0, 1):
                                c0 = j * FD + h * 512
                                nc.tensor.matmul(
                                    pz[:, h * 512 : (h + 1) * 512], Ident,
                                    pslot(i)[:, c0 : c0 + 512],
                                    start=first_h[h], stop=False,
                                    skip_group_check=True,
                                )
                                first_h[h] = False
                    if i == 0:
                        te.wait_ge(obasem, 16)
                    elif i == 4:
                        te.wait_ge(obbsem, 16)
                    for h in (0, 1):
                        mm = nc.tensor.matmul(
                            pz[:, h * 512 : (h + 1) * 512], btsl(i), OHD,
                            start=False, stop=True, skip_group_check=True,
                        )
                        if h == 1:
                            mm.then_inc(zsem, 1)
                if i >= WLAG:
                    rr = i - WLAG
                    if WLPOOL:
                        te.wait_ge(psem, rr + 1)
                    else:
                        te.wait_ge(vsem, C[("wl", rr)])
                    wl = wlslot(rr)
                    nc.tensor.matmul(
                        ps0[:, :], OH3[:, rr, :], wl[:, 0:512],
                        start=(rr == 0), stop=False, skip_group_check=True,
                    )
                    nc.tensor.matmul(
                        ps1[:, :], OH3[:, rr, :], wl[:, 512:1024],
                        start=(rr == 0), stop=False, skip_group_check=True,
                    ).then_inc(msem, 1)
                    if rr == 0:
                        # accumulate the L*b_v rows early (order irrelevant)
                        nc.tensor.matmul(
                            ps0[:, :], lbvT, OHD,
                            start=False, stop=False, skip_group_check=True,
                        )
                        nc.tensor.matmul(
                            ps1[:, :], lbvT, OHD,
                            start=False, stop=False, skip_group_check=True,
                        )
            for rr in range(RC - WLAG, RC - NTAIL):
                if WLPOOL:
                    te.wait_ge(psem, rr + 1)
                else:
                    te.wait_ge(vsem, C[("wl", rr)])
                wl = wlslot(rr)
                nc.tensor.matmul(
                    ps0[:, :], OH3[:, rr, :], wl[:, 0:512],
                    start=False, stop=False, skip_group_check=True,
                )
                nc.tensor.matmul(
                    ps1[:, :], OH3[:, rr, :], wl[:, 512:1024],
                    start=False, stop=False, skip_group_check=True,
                ).then_inc(msem, 1)
            if NTAIL >= 2:
                rr = RC - 2
                te.wait_ge(vsem, C[("stt", rr)])
                wl = wlslot(rr)
                nc.tensor.matmul(
                    ps0[:, :], OH3[:, rr, :], wl[:, 0:512],
                    start=False, stop=False, skip_group_check=True,
                )
                nc.tensor.matmul(
                    ps1[:, :], OH3[:, rr, :], wl[:, 512:1024],
                    start=False, stop=False, skip_group_check=True,
                ).then_inc(msem, 1)
            rr = RC - 1
            wl = wlslot(rr)
            te.wait_ge(vsem, C[("stta", rr)])
            nc.tensor.matmul(
                ps0[:, :], OH3[:, rr, :], wl[:, 0:512],
                start=False, stop=True, skip_group_check=True,
            ).then_inc(msem, 1)
            te.wait_ge(vsem, C[("sttb", rr)])
            nc.tensor.matmul(
                ps1[:, :], OH3[:, rr, :], wl[:, 512:1024],
                start=False, stop=True, skip_group_check=True,
            ).then_inc(msem, 1)

    return nc


_CACHE = {}


def _buckets(a):
    """Deal r-indices into NK buckets of RC; zeros occupy each bucket's
    fast-mask positions first. Returns (buckets, nfast)."""
    zeros = [r for r in range(R) if a[r] == 0.0]
    pos = [r for r in range(R) if a[r] != 0.0]
    zbuck = [[] for _ in range(NK)]
    for j, r in enumerate(zeros):
        zbuck[j % NK].append(r)
    nfast = min(min(len(zb) for zb in zbuck), RC)
    mask = _fast_mask(nfast)
    pi = 0
    buckets = []
    for k in range(NK):
        zq = list(zbuck[k])
        rl = [None] * RC
        for i in range(RC):
            if mask[i]:
                rl[i] = zq.pop(0)
        for i in range(RC):
            if rl[i] is None:
                if zq:
                    rl[i] = zq.pop(0)
                else:
                    rl[i] = pos[pi]
                    pi += 1
        buckets.append(rl)
    return buckets, nfast


def _prepare(X, T, M, PD, alpha, w_v, w_t, b_t, b_v, ref_time):
    """Pack full inputs into per-core DRAM parameter maps.
    Returns (mask, buckets, in_maps)."""
    a = np.maximum(alpha.reshape(R), 0.0)
    s_ref = ref_time.reshape(R)
    nas = -(a * s_ref)
    bt4 = 4.0 * b_t[..., 0]              # [R, L, D]
    lbv = float(L) * b_v[:, 0, :]        # [R, D]

    buckets, nfast = _buckets(a)
    mask = _fast_mask(nfast)

    # per-r params: [w1|w0|w2|w3|wv] (5*D per iter)
    wts = np.stack(
        [w_t[..., 1], w_t[..., 0], w_t[..., 2], w_t[..., 3], w_v], axis=2
    )                                     # [R, L, 5, D]

    oh = np.zeros((L, RC, RC), np.float32)
    for r in range(RC):
        oh[:, r, r] = 1.0
    ident = np.eye(L, dtype=np.float32)
    ohd = np.zeros((64, 512), np.float32)
    for b in range(8):
        for d in range(64):
            ohd[d, b * 64 + d] = 1.0

    cstf = np.zeros((L, CONST_W), np.float32)
    cstf[:, 0 : RC * RC] = oh.reshape(L, RC * RC)
    cstf[:, RC * RC :] = ident
    cst = cstf.astype(_nbf16)

    in_maps = []
    for c in range(8):
        b0 = (c // NK) * BC
        rl = buckets[c % NK]
        tr = lambda x: np.ascontiguousarray(
            x[b0 : b0 + BC].transpose(1, 0, 2).reshape(L, FD)
        )
        cfc = np.zeros((L, CFC_W), np.float32)
        cfc[:, 0:RC] = a[rl]
        cfc[:, RC : 2 * RC] = nas[rl]
        cbf = np.zeros((L, CB_W), np.float32)
        trx = tr(X)
        cbf[:, XH0E : XH0E + 512] = trx[:, 0:512]
        cbf[:, XH1E : XH1E + 512] = trx[:, 512:FD]
        cbf[:, MTE : MTE + FD] = tr(M)
        cbf[:, PDE : PDE + FD] = tr(PD)
        for i, r in enumerate(rl):
            if i < 2:
                base = W01E + i * WS_W
            elif i < 6:
                base = W25E + (i - 2) * WS_W
            else:
                base = W611E + (i - 6) * WS_W
            cbf[:, base : base + WS_W] = wts[r].reshape(L, WS_W)
        oba = np.zeros((64, OBA_W), np.float32)
        oba[:, 0:512] = ohd
        obb = np.zeros((64, OBB_W), np.float32)
        for i, r in enumerate(rl):
            oba[0:D, 512 + i] = lbv[r]
            if i < 4:
                oba[0:D, 512 + 16 + i * L : 512 + 16 + (i + 1) * L] = bt4[r].T
            else:
                obb[0:D, (i - 4) * L : (i - 3) * L] = bt4[r].T
        in_maps.append(
            {
                "cfc": cfc,
                "cfT": tr(T).astype(_nbf16),
                "cb": np.ascontiguousarray(cbf).astype(_nbf16),
                "CST": cst,
                "OBA": oba.astype(_nbf16),
                "OBB": obb.astype(_nbf16),
            }
        )
    return mask, buckets, in_maps


def kernel(X, T, M, PD, alpha, w_v, w_t, b_t, b_v, ref_time):
    X = np.asarray(X, np.float32)
    T = np.asarray(T, np.float32)
    M = np.asarray(M, np.float32)
    PD = np.asarray(PD, np.float32)
    alpha = np.asarray(alpha, np.float32)
    w_v = np.asarray(w_v, np.float32)
    w_t = np.asarray(w_t, np.float32)
    b_t = np.asarray(b_t, np.float32)
    b_v = np.asarray(b_v, np.float32)
    ref_time = np.asarray(ref_time, np.float32)

    mask, buckets, in_maps = _prepare(
        X, T, M, PD, alpha, w_v, w_t, b_t, b_v, ref_time
    )

    if mask not in _CACHE:
        _CACHE[mask] = _build_graph(mask)
    nc = _CACHE[mask]

    trace = bool(os.environ.get("BASS_KERNEL_TRACE"))
    kw = {}
    if trace:
        tmpdir = os.environ.get("BASS_KERNEL_TRACE_DIR") or None
        kw = dict(trace=True, tmpdir=tmpdir)
    for _attempt in range(3):
        res = run_bass_kernel_spmd(nc, in_maps, core_ids=list(range(8)), **kw)
        outs = [
            np.asarray(res.results[c]["out"], np.float32) for c in range(8)
        ]
        if all(np.isfinite(o).all() for o in outs):
            break
    if trace:
        _CACHE["exec_time_ns"] = res.exec_time_ns
        print(f"HW exec time: {res.exec_time_ns} ns")

    out = np.zeros((B, R, D), np.float32)
    for c in range(8):
        b0 = (c // NK) * BC
        rl = buckets[c % NK]
        o = outs[c].reshape(RC, BC, D)
        for i, r in enumerate(rl):
            out[b0 : b0 + BC, r] = o[i]
    return out


# revision 48
# speedup vs baseline: 1.0461x; 1.0342x over previous
"""ALNN layer kernel for 8 TRN2 NeuronCores (raw Bass, explicit semaphores).

out[b,r,d] = relu( sum_l w_v[r,l,d]*relu(z[b,r,l,d]) + L*b_v[r,d] )
z = wt0*X + wt1*relu(X)*k + wt2*M + wt3*PD + 4*bt
k = exp(-relu(alpha_r)*|T - s_r|)        (uses relu(X*k) == relu(X)*k, k>0)

Sharding: B split 2 ways x R dealt into 4 buckets -> 8 cores, 16 b x 12 r
each. Cores c and c+4 share bucket c%4. r's with relu(alpha)=0 take a
fast path (k == 1). The graph is SPMD-uniform: a shared fast-mask sized
by the min zero-count across buckets.

v8 (from v7 @ 60816ns -> ~56.5us measured): DVE-bound design.
 - DVE is the saturated engine (~40.5us): products merged into ONE
   quad op [q|m0|m2|m3] per fast iter (FD=4096 @2x_1p; tensor_tensor
   is capped at 2x — no 4x uop) or tri+g+qs (slow). wl = lat*wv also
   on DVE (gpsimd tensor ops are ~2x slower than modeled AND slow
   concurrent DVE ops ~4x via SBUF contention — do not use).
 - Slow slots interleaved at 3,5,7,9 (SLOW_FILL) so ACT-heavy abs/exp
   iters alternate with DVE-heavy slow-product iters; abs/exp pair j
   emitted at ACT iter slows[j]-1.
 - PE: z via identity matmuls (one vsem wait per DVE op), bt via
   btsl x OHD trick, out-reduce via one-hot; ~14 warmup matmuls hold
   the p-state until real work. Every psum column's first write needs
   start=True; reading a psum bank mid-accumulation-group faults HW.
 - ACT: relu + slow-iter abs/exp; a dummy activation right after the
   cfc wait forces the lazy ~1.3us ACT_TABLE_LOAD off the critical
   path. Final relus read ps0/ps1 only after msem 12/13 (groups
   closed); output as bf16, column-split DMAs from the sync ring.
 - DMA: ~7.3us fixed framework preamble, then ~230-260GB/s aggregate
   over 16 engines shared by all rings. Consumption-ordered chunks:
   sync ring [Xt-h0 | W0W1 | Xt-h1], deferred [W2-5 | W6-11] after
   s3; ACT ring cfc, CST, Mt, PDt; gpsimd SWDGE ring OBA, then
   (deferred) cfT, OBB. Same-ring DMA completion is FIFO per engine,
   so a later chunk's semaphore implies earlier chunks landed.
 - Tail: last iter's quad split into column halves feeding the PE's
   half-closed (z2sem) last z-group; wl via scalar_tensor_tensor
   straight from PSUM; NaN-retry guard for a rare (~1/20) transient.

Raw bass: at most ONE attached sync-wait per compute instruction, so
cross-engine deps use standalone wait_ge; each DMA gets a dedicated
semaphore (two DMAs sharing one sem can interleave per-queue completions,
so a partial wait would be unsound).
"""

import os
import numpy as np
import ml_dtypes

import concourse.bass as bass
import concourse.mybir as mybir
from concourse.bass_utils import run_bass_kernel_spmd

AF = mybir.ActivationFunctionType
OP = mybir.AluOpType
BF16 = mybir.dt.bfloat16
F32 = mybir.dt.float32

B, R, L, D = 32, 48, 128, 64
NB, NK = 2, 4              # b-halves x r-buckets = 8 cores
BC, RC = B // NB, R // NK  # 16 b, 12 r per core
FD = BC * D                # 1024 free elems

CFC_W = 2 * RC             # f32 consts: [Abc 12 | NASbc 12]
WS_W = 5 * D               # per-iter param slice: [w1|w0|w2|w3|wv]
STRD = FD + 2 * WS_W       # 1664: uniform stride of the data planes

# BIG sbuf layout (bf16), stride-1664 planes:
XPC = 0                    # Xp   [0:1024]
WSCC = 1024                # wsc  [1024:1536] (warmup ones), pad to 1664
XC = STRD                  # Xt   [1664:2688] | pad [2688:3328]
MC = 2 * STRD              # Mt   [3328:4352] | pad
PDC = 3 * STRD             # PDt  [4992:6016]
W0C = PDC + FD             # W0..W11 [6016:9856]
GC = W0C + 12 * WS_W       # g    [9856:10880]
BIG_W = GC + FD            # 10880

# DRAM cb stream, consumption order:
# [Xt-h0 | W0 W1 | Xt-h1 | Mt | PDt | W2-5 | W6-11]
CB_W = 6912
XH0E = 0
W01E = 512
XH1E = W01E + 2 * WS_W     # 1152
MTE = XH1E + 512           # 1664
PDE = MTE + FD             # 2688
W25E = PDE + FD            # 3712
W611E = W25E + 4 * WS_W    # 4992

CONST_W = RC * RC + L      # [OH3 144 | Ident 128]
OBA_W = 512 + 16 + 4 * L   # [OHD | lbvT(16 pad) | BT0-3]
OBB_W = 8 * L              # [BT4-11]


def wcol(i):
    return W0C + i * WS_W

_nbf16 = ml_dtypes.bfloat16

WLPOOL = False  # gpsimd tensor ops are slow and poison concurrent DVE ops

LB = 5    # wl buffers (Pool/DVE -> PE)
LAB = 4   # lat buffers (ACT -> Pool)
PRB = 3   # product buffer sets (DVE -> PE)
ZB = 3    # psum z triple-buffer (6 of 8 banks; ps0/ps1 take the rest)
WLAG = 3  # out-mm for rr = i - WLAG emitted in PE iter i
ALAG = 2  # relu for rr = i - ALAG emitted in ACT iter i
NWARM = 14  # PE warmup matmuls (keep PE out of low p-state until work)
NSPLIT = 2  # iters with split product ops (partial-input start)
NTAIL = 1   # last iters: wl via DVE stt straight from PSUM


SLOW_FILL = (3, 5, 7, 9, 2, 4, 6, 8, 1, 10, 0, 11)


def _fast_mask(nfast):
    """Interleave slow slots (odd-first) so ACT-heavy abs/exp iterations
    alternate with DVE-heavy slow-product iterations; head and tail slots
    stay fast when possible (early start, short drain)."""
    nslow = RC - nfast
    mask = [True] * RC
    for s in SLOW_FILL[:nslow]:
        mask[s] = False
    return tuple(mask)


def _dve_schedule(mask):
    """DVE emission order; must match the @block.vector body exactly."""
    sched = []
    for i in range(RC):
        if i == 0 and mask[0] and NSPLIT > 0:
            sched.append(("p1h0", 0))
            sched.append(("p1h1", 0))
        if i < NSPLIT:
            if mask[i]:
                if i > 0:
                    sched.append(("p1", i))    # [q|m0]
            else:
                sched.append(("m0s", i))
            sched.append(("p2", i))        # [m2|m3]
            if not mask[i]:
                sched.append(("g", i))
                sched.append(("qs", i))
        else:
            if mask[i]:
                if i == RC - 1:
                    if RC - 1 >= WLAG - 1:
                        sched.append(("wl", RC - 1 - (WLAG - 1)))
                    sched.append(("qh0", i))
                    sched.append(("qh1", i))
                else:
                    sched.append(("quad", i))  # [q|m0|m2|m3]
            else:
                sched.append(("tri", i))   # [m0|m2|m3]
                sched.append(("g", i))
                sched.append(("qs", i))
        if not WLPOOL and i >= WLAG - 1 and not (
            i == RC - 1 and mask[i] and i >= NSPLIT
        ):
            rr = i - (WLAG - 1)
            if rr <= RC - NTAIL - 1:
                sched.append(("wl", rr))
    if not WLPOOL:
        for rr in range(RC - (WLAG - 1), RC - NTAIL):
            sched.append(("wl", rr))
    if NTAIL >= 2:
        sched.append(("stt", RC - 2))
    sched.append(("stta", RC - 1))
    sched.append(("sttb", RC - 1))
    return sched


def _build_graph(mask, detect_races=True):
    nslow = sum(1 for f in mask if not f)
    ksl = max(nslow, 1)
    slows = [i for i in range(RC) if not mask[i]]
    kidx = {i: j for j, i in enumerate(slows)}  # slow iter -> k slot
    sched = _dve_schedule(mask)
    C = {key: idx + 1 for idx, key in enumerate(sched)}

    nc = bass.Bass(detect_race_conditions=detect_races)
    cfc_e = nc.declare_dram_parameter("cfc", [L, CFC_W], F32, isOutput=False)
    cft_e = nc.declare_dram_parameter("cfT", [L, FD], BF16, isOutput=False)
    cb_e = nc.declare_dram_parameter("cb", [L, CB_W], BF16, isOutput=False)
    cst_e = nc.declare_dram_parameter("CST", [L, CONST_W], BF16, isOutput=False)
    oba_e = nc.declare_dram_parameter("OBA", [64, OBA_W], BF16, isOutput=False)
    obb_e = nc.declare_dram_parameter("OBB", [64, OBB_W], BF16, isOutput=False)
    out_e = nc.declare_dram_parameter("out", [RC, FD], BF16, isOutput=True)

    from contextlib import ExitStack

    with ExitStack() as ctx:
        e = ctx.enter_context
        cfc = e(nc.sbuf_tensor([L, CFC_W], F32))
        cft = e(nc.sbuf_tensor([L, FD], BF16))
        BIG = e(nc.sbuf_tensor([L, BIG_W], BF16))
        CST = e(nc.sbuf_tensor([L, CONST_W], BF16))
        OBA = e(nc.sbuf_tensor([64, OBA_W], BF16))
        OBB = e(nc.sbuf_tensor([64, OBB_W], BF16))
        dist = e(nc.sbuf_tensor([L, FD], F32))
        kbuf = e(nc.sbuf_tensor([L, ksl * FD], BF16))
        prods = e(nc.sbuf_tensor([L, PRB * 4 * FD], BF16))
        latb = e(nc.sbuf_tensor([L, LAB * FD], BF16))
        wlbuf = e(nc.sbuf_tensor([L, LB * FD], BF16))
        outt = e(nc.sbuf_tensor([RC, FD], BF16))
        psz = [e(nc.psum_tensor(f"psz{j}", [L, FD], F32)) for j in range(ZB)]
        ps0 = e(nc.psum_tensor([RC, 512], F32))
        ps1 = e(nc.psum_tensor([RC, 512], F32))
        s1sem = e(nc.semaphore("s1sem"))    # Xt-h0
        s1bsem = e(nc.semaphore("s1bsem"))  # Xt-h1
        swsem = e(nc.semaphore("swsem"))    # W0|W1
        s2sem = e(nc.semaphore("s2sem"))    # cb c2: Mt|W2|W3
        s3sem = e(nc.semaphore("s3sem"))    # cb c3: PDt|W4|W5
        s4sem = e(nc.semaphore("s4sem"))    # cb c4: W2-5
        s5sem = e(nc.semaphore("s5sem"))    # cb c5: W6-11
        cfcsem = e(nc.semaphore("cfcsem"))
        cstsem = e(nc.semaphore("cstsem"))  # OH3|Ident
        cftsem = e(nc.semaphore("cftsem"))
        obasem = e(nc.semaphore("obasem"))  # OHD|lbvT|BT0-3
        obbsem = e(nc.semaphore("obbsem"))  # BT4-11
        asem = e(nc.semaphore("asem"))      # ACT k completions
        lsem = e(nc.semaphore("lsem"))      # ACT lat relu completions
        psem = e(nc.semaphore("psem"))      # Pool wl completions
        zsem = e(nc.semaphore("zsem"))      # PE z-group completions (1/iter)
        z2sem = e(nc.semaphore("z2sem"))    # PE last z-group half completions
        msem = e(nc.semaphore("msem"))      # PE out-mm completions
        vsem = e(nc.semaphore("vsem"))      # DVE op completions
        osem = e(nc.semaphore("osem"))
        rsem = e(nc.semaphore("rsem"))
        gsem = e(nc.semaphore("gsem"))
        xsem = e(nc.semaphore("xsem"))  # ACT-computed Xp halves
        block = e(nc.Block())

        Abc = cfc[:, 0:RC]
        NASbc = cfc[:, RC : 2 * RC]
        Xp = BIG[:, XPC : XPC + FD]
        Xt = BIG[:, XC : XC + FD]
        Mt = BIG[:, MC : MC + FD]
        PDt = BIG[:, PDC : PDC + FD]
        gbuf = BIG[:, GC : GC + FD]
        wsc = BIG[:, WSCC : WSCC + 512]
        OH3 = CST[:, 0 : RC * RC].rearrange("p (r m) -> p r m", r=RC)
        Ident = CST[:, RC * RC : RC * RC + L]
        OHD = OBA[:, 0:512]
        lbvT = OBA[:, 512 : 512 + RC]

        def r3(ap):
            return ap.rearrange("p (b d) -> p b d", b=BC)

        def kslot(j):
            return kbuf[:, j * FD : (j + 1) * FD]

        def wbc(i, ei):
            base = wcol(i) + ei * D
            return BIG[:, base : base + D].unsqueeze(1).broadcast_to([L, BC, D])

        def wbch(i, ei):
            base = wcol(i) + ei * D
            return (
                BIG[:, base : base + D].unsqueeze(1).broadcast_to([L, BC // 2, D])
            )

        def wgrp(i, e0, n):
            """weights slots e0..e0+n-1 of iter i: [L, n, 16, 64] bcast."""
            base = wcol(i) + e0 * D
            return (
                BIG[:, base : base + n * D]
                .rearrange("p (a d) -> p a d", a=n)
                .unsqueeze(2)
                .broadcast_to([L, n, BC, D])
            )

        def dgrp(c0, n):
            """n data planes at stride STRD from col c0: [L, n, 16, 64]."""
            return (
                BIG[:, c0 : c0 + n * STRD]
                .rearrange("p (a c) -> p a c", a=n)[:, :, 0:FD]
                .rearrange("p a (b d) -> p a b d", b=BC)
            )

        def pslot(i):
            return prods[:, (i % PRB) * 4 * FD : (i % PRB + 1) * 4 * FD]

        def pview(i, e0, n):
            s = pslot(i)
            return s[:, e0 * FD : (e0 + n) * FD].rearrange(
                "p (a b d) -> p a b d", a=n, b=BC
            )

        def btsl(i):
            if i < 4:
                return OBA[:, 512 + 16 + i * L : 512 + 16 + (i + 1) * L]
            return OBB[:, (i - 4) * L : (i - 3) * L]

        def latslot(rr):
            return latb[:, (rr % LAB) * FD : (rr % LAB + 1) * FD]

        def wlslot(rr):
            return wlbuf[:, (rr % LB) * FD : (rr % LB + 1) * FD]

        @block.sync
        def _(sp):
            sp.dma_start(
                out=BIG[:, XC : XC + 512], in_=cb_e[:, XH0E : XH0E + 512]
            ).then_inc(s1sem, 16)
            sp.dma_start(
                out=BIG[:, W0C : W0C + 2 * WS_W],
                in_=cb_e[:, W01E : W01E + 2 * WS_W],
            ).then_inc(swsem, 16)
            sp.dma_start(
                out=BIG[:, XC + 512 : XC + FD], in_=cb_e[:, XH1E : XH1E + 512]
            ).then_inc(s1bsem, 16)
            sp.wait_ge(s3sem, 16)
            sp.dma_start(
                out=BIG[:, W0C + 2 * WS_W : W0C + 6 * WS_W],
                in_=cb_e[:, W25E : W25E + 4 * WS_W],
            ).then_inc(s4sem, 16)
            sp.dma_start(
                out=BIG[:, W0C + 6 * WS_W : GC], in_=cb_e[:, W611E:]
            ).then_inc(s5sem, 16)
            sp.wait_ge(rsem, 1)
            sp.dma_start(out=out_e[:, 0:512], in_=outt[:, 0:512]).then_inc(
                osem, 16
            )
            sp.wait_ge(rsem, 2)
            sp.dma_start(out=out_e[:, 512:], in_=outt[:, 512:]).then_inc(
                osem, 16
            )

        @block.scalar
        def _(act):
            act.dma_start(out=cfc[:, :], in_=cfc_e[:, :]).then_inc(cfcsem, 16)
            act.dma_start(out=CST[:, :], in_=cst_e[:, :]).then_inc(cstsem, 16)
            act.dma_start(
                out=BIG[:, MC : MC + FD], in_=cb_e[:, MTE : MTE + FD]
            ).then_inc(s2sem, 16)
            act.dma_start(
                out=BIG[:, PDC : PDC + FD], in_=cb_e[:, PDE : PDE + FD]
            ).then_inc(s3sem, 16)
            # dummy op: forces the lazy ACT_TABLE_LOAD (~1.3us) to happen
            # now, while ACT is idle, instead of on the Xp critical path.
            # wsc (Pool memset, gsem) is ready well before cfc's DMA lands.
            act.wait_ge(gsem, 1)
            nc.scalar.activation(
                dist[0:1, 0:1], BIG[0:1, WSCC : WSCC + 1], AF.Relu
            )
            # Xp = relu(Xt) on the idle ACT engine (frees ~0.6us of DVE)
            act.wait_ge(s1sem, 16)
            nc.scalar.activation(Xp[:, 0:512], Xt[:, 0:512], AF.Relu).then_inc(
                xsem, 1
            )
            act.wait_ge(s1bsem, 16)
            nc.scalar.activation(Xp[:, 512:FD], Xt[:, 512:FD], AF.Relu).then_inc(
                xsem, 1
            )
            pj = [max(1, s - 1) for s in slows]
            cft_waited = False
            for i in range(RC):
                if ALAG <= i and i - ALAG <= RC - NTAIL - 1:
                    rr = i - ALAG
                    act.wait_ge(zsem, rr + 1)
                    if rr >= LAB:
                        if WLPOOL:
                            act.wait_ge(psem, rr - LAB + 1)
                        else:
                            act.wait_ge(vsem, C[("wl", rr - LAB)])
                    nc.scalar.activation(
                        latslot(rr), psz[rr % ZB][:, :], AF.Relu
                    ).then_inc(lsem, 1)
                for j in range(nslow):
                    if pj[j] != i:
                        continue
                    if not cft_waited:
                        act.wait_ge(cfcsem, 16)
                        act.wait_ge(cftsem, 16)
                        cft_waited = True
                    si = slows[j]
                    nc.scalar.activation(
                        dist[:, :], cft[:, :], AF.Abs,
                        bias=NASbc[:, si : si + 1], scale=Abc[:, si : si + 1],
                    )
                    nc.scalar.activation(
                        kslot(j), dist[:, :], AF.Exp, scale=-1.0
                    ).then_inc(asem, 1)
            # drain the output in two row chunks: rows rr are final right
            # after iteration rr's out-mm (one-hot row writes)
            for rr in range(RC - ALAG, RC - NTAIL):
                act.wait_ge(zsem, rr + 1)
                act.wait_ge(vsem, C[("wl", rr - LAB)])
                nc.scalar.activation(
                    latslot(rr), psz[rr % ZB][:, :], AF.Relu
                ).then_inc(lsem, 1)
            act.wait_ge(msem, 12)
            nc.scalar.activation(outt[:, 0:512], ps0[:, :], AF.Relu).then_inc(
                rsem, 1
            )
            act.wait_ge(msem, 13)
            nc.scalar.activation(outt[:, 512:], ps1[:, :], AF.Relu).then_inc(
                rsem, 1
            )

        def halfop(i, e0, n, h, in0c):
            c0 = h * 512
            in0h = (
                BIG[:, in0c : in0c + n * STRD]
                .rearrange("p (a c) -> p a c", a=n)[:, :, c0 : c0 + 512]
                .rearrange("p a (b d) -> p a b d", b=BC // 2)
            )
            wh = (
                BIG[:, wcol(i) + e0 * D : wcol(i) + (e0 + n) * D]
                .rearrange("p (a d) -> p a d", a=n)
                .unsqueeze(2)
                .broadcast_to([L, n, BC // 2, D])
            )
            outh = (
                pslot(i)[:, e0 * FD : (e0 + n) * FD]
                .rearrange("p (a c) -> p a c", a=n)[:, :, c0 : c0 + 512]
                .rearrange("p a (b d) -> p a b d", b=BC // 2)
            )
            return nc.vector.tensor_tensor(outh, in0h, wh, OP.mult)

        @block.vector
        def _(ve):
            w2seen = w3seen = w4seen = w5seen = False
            for i in range(RC):
                if i == 0:
                    if mask[0] and NSPLIT > 0:
                        ve.wait_ge(xsem, 1)
                        ve.wait_ge(swsem, 16)
                        w2seen = True
                        halfop(0, 0, 2, 0, XPC).then_inc(vsem, 1)
                        ve.wait_ge(xsem, 2)
                        halfop(0, 0, 2, 1, XPC).then_inc(vsem, 1)
                    else:
                        ve.wait_ge(xsem, 2)
                if i >= PRB:
                    ve.wait_ge(zsem, i - PRB + 1)
                if i < NSPLIT:
                    if not w2seen:
                        ve.wait_ge(swsem, 16)
                        w2seen = True
                    if mask[i]:
                        if i > 0:
                            # [q|m0] <- (Xp,Xt)*(w1,w0)
                            nc.vector.tensor_tensor(
                                pview(i, 0, 2), dgrp(XPC, 2), wgrp(i, 0, 2),
                                OP.mult,
                            ).then_inc(vsem, 1)
                    else:
                        nc.vector.tensor_tensor(
                            r3(pslot(i)[:, FD : 2 * FD]), r3(Xt), wbc(i, 1),
                            OP.mult,
                        ).then_inc(vsem, 1)
                    if not w3seen:
                        ve.wait_ge(s3sem, 16)
                        w3seen = True
                    # [m2|m3] <- (Mt,PDt)*(w2,w3)
                    nc.vector.tensor_tensor(
                        pview(i, 2, 2), dgrp(MC, 2), wgrp(i, 2, 2), OP.mult
                    ).then_inc(vsem, 1)
                else:
                    if not w4seen:
                        ve.wait_ge(s4sem, 16)
                        w4seen = True
                    if i >= 6 and not w5seen:
                        ve.wait_ge(s5sem, 16)
                        w5seen = True
                    if mask[i]:
                        if i == RC - 1:
                            rr = i - (WLAG - 1)
                            ve.wait_ge(lsem, rr + 1)
                            if rr >= LB:
                                ve.wait_ge(msem, rr - LB + 1)
                            nc.vector.tensor_tensor(
                                r3(wlslot(rr)), r3(latslot(rr)), wbc(rr, 4),
                                OP.mult,
                            ).then_inc(vsem, 1)
                            # halved quad: h0 then h1 so the PE's last
                            # z-group h0 overlaps the h1 product
                            for h in (0, 1):
                                c0 = h * 512
                                in0h = (
                                    BIG[:, 0 : 4 * STRD]
                                    .rearrange("p (a c) -> p a c", a=4)[
                                        :, :, c0 : c0 + 512
                                    ]
                                    .rearrange(
                                        "p a (b d) -> p a b d", b=BC // 2
                                    )
                                )
                                wh = (
                                    BIG[:, wcol(i) : wcol(i) + 4 * D]
                                    .rearrange("p (a d) -> p a d", a=4)
                                    .unsqueeze(2)
                                    .broadcast_to([L, 4, BC // 2, D])
                                )
                                outh = (
                                    pslot(i)
                                    .rearrange("p (a c) -> p a c", a=4)[
                                        :, :, c0 : c0 + 512
                                    ]
                                    .rearrange(
                                        "p a (b d) -> p a b d", b=BC // 2
                                    )
                                )
                                nc.vector.tensor_tensor(
                                    outh, in0h, wh, OP.mult
                                ).then_inc(vsem, 1)
                        else:
                            # [q|m0|m2|m3] <- (Xp,Xt,Mt,PDt)*(w1,w0,w2,w3)
                            nc.vector.tensor_tensor(
                                pview(i, 0, 4), dgrp(XPC, 4), wgrp(i, 0, 4),
                                OP.mult,
                            ).then_inc(vsem, 1)
                    else:
                        # [m0|m2|m3] <- (Xt,Mt,PDt)*(w0,w2,w3)
                        nc.vector.tensor_tensor(
                            pview(i, 1, 3), dgrp(XC, 3), wgrp(i, 1, 3), OP.mult
                        ).then_inc(vsem, 1)
                if not mask[i]:
                    ve.wait_ge(asem, kidx[i] + 1)
                    nc.vector.tensor_mul(
                        gbuf[:, :], Xp[:, :], kslot(kidx[i])
                    ).then_inc(vsem, 1)
                    nc.vector.tensor_tensor(
                        r3(pslot(i)[:, 0:FD]), r3(gbuf[:, :]), wbc(i, 0),
                        OP.mult,
                    ).then_inc(vsem, 1)
                if not WLPOOL and i >= WLAG - 1 and not (
                    i == RC - 1 and mask[i] and i >= NSPLIT
                ):
                    rr = i - (WLAG - 1)
                    if rr <= RC - NTAIL - 1:
                        ve.wait_ge(lsem, rr + 1)
                        if rr >= LB:
                            ve.wait_ge(msem, rr - LB + 1)
                        nc.vector.tensor_tensor(
                            r3(wlslot(rr)), r3(latslot(rr)), wbc(rr, 4),
                            OP.mult,
                        ).then_inc(vsem, 1)
            if not WLPOOL:
                for rr in range(RC - (WLAG - 1), RC - NTAIL):
                    ve.wait_ge(lsem, rr + 1)
                    ve.wait_ge(msem, rr - LB + 1)
                    nc.vector.tensor_tensor(
                        r3(wlslot(rr)), r3(latslot(rr)), wbc(rr, 4), OP.mult
                    ).then_inc(vsem, 1)
            # fused relu*wv straight from PSUM for the last iteration(s);
            # the very last one in halves so the out matmuls/relus pipeline
            if NTAIL >= 2:
                rr = RC - 2
                ve.wait_ge(zsem, rr + 1)
                ve.wait_ge(msem, rr - LB + 1)
                nc.vector.scalar_tensor_tensor(
                    r3(wlslot(rr)), r3(psz[rr % ZB][:, :]), 0.0, wbc(rr, 4),
                    OP.max, OP.mult,
                ).then_inc(vsem, 1)
            rr = RC - 1
            ve.wait_ge(msem, rr - LB + 1)
            for h, zwait in ((0, 1), (1, 2)):
                ve.wait_ge(z2sem, zwait)
                c0, c1 = h * 512, (h + 1) * 512
                wl3 = wlslot(rr)[:, c0:c1].rearrange("p (b d) -> p b d", b=BC // 2)
                pz3 = psz[rr % ZB][:, c0:c1].rearrange(
                    "p (b d) -> p b d", b=BC // 2
                )
                nc.vector.scalar_tensor_tensor(
                    wl3, pz3, 0.0, wbch(rr, 4), OP.max, OP.mult
                ).then_inc(vsem, 1)

        @block.gpsimd
        def _(gp):
            nc.gpsimd.memset(wsc[:, :], 1.0).then_inc(gsem, 1)
            gp.dma_start(out=OBA[:, :], in_=oba_e[:, :]).then_inc(obasem, 16)
            gp.wait_ge(obasem, 16)
            gp.dma_start(out=cft[:, :], in_=cft_e[:, :]).then_inc(cftsem, 16)
            gp.dma_start(out=OBB[:, :], in_=obb_e[:, :]).then_inc(obbsem, 16)
            for rr in range(RC - NTAIL if WLPOOL else 0):
                gp.wait_ge(lsem, rr + 1)
                if rr >= LB:
                    gp.wait_ge(msem, rr - LB + 1)
                nc.gpsimd.tensor_tensor(
                    r3(wlslot(rr)), r3(latslot(rr)), wbc(rr, 4), OP.mult
                ).then_inc(psem, 1)

        @block.tensor
        def _(te):
            # warmup: keep the PE out of its low p-state until real work
            # arrives. Results never read; ps0 reset by the real start=True.
            te.wait_ge(gsem, 1)
            for _w in range(NWARM):
                nc.tensor.matmul(
                    ps0[:, :], wsc[:, 0:RC], wsc[:, :],
                    start=True, stop=True, skip_group_check=True,
                )
            te.wait_ge(cstsem, 16)
            for i in range(RC):
                last = i == RC - 1
                if i >= ZB and i - ZB <= RC - NTAIL - 1:
                    te.wait_ge(lsem, i - ZB + 1)
                pz = psz[i % ZB]
                # (wait_key, first_prod_slot, nprods) groups in psum order
                if i == 0 and mask[0] and NSPLIT > 0:
                    groups = [(("p1h0", 0), "h0", 2), (("p1h1", 0), "h1", 2),
                              (("p2", 0), 2, 2)]
                elif i < NSPLIT:
                    if mask[i]:
                        groups = [(("p1", i), 0, 2), (("p2", i), 2, 2)]
                    else:
                        groups = [(("m0s", i), 1, 1), (("p2", i), 2, 2),
                                  (("qs", i), 0, 1)]
                elif mask[i]:
                    groups = [(("quad", i), 0, 4)]
                else:
                    groups = [(("tri", i), 1, 3), (("qs", i), 0, 1)]
                if last:
                    # h0 stream first, then h1, each closed separately so the
                    # DVE's stt halves overlap with this group's tail
                    fast_last = mask[i] and i >= NSPLIT
                    for h in (0, 1):
                        first = True
                        if fast_last:
                            te.wait_ge(vsem, C[(f"qh{h}", i)])
                        for key, e0, n in groups:
                            if h == 0 and not fast_last:
                                te.wait_ge(vsem, C[key])
                            for j in range(e0, e0 + n):
                                c0 = j * FD + h * 512
                                nc.tensor.matmul(
                                    pz[:, h * 512 : (h + 1) * 512], Ident,
                                    pslot(i)[:, c0 : c0 + 512],
                                    start=first, stop=False,
                                    skip_group_check=True,
                                )
                                first = False
                        nc.tensor.matmul(
                            pz[:, h * 512 : (h + 1) * 512], btsl(i), OHD,
                            start=False, stop=True, skip_group_check=True,
                        ).then_inc(z2sem, 1)
                else:
                    first_h = [True, True]
                    for key, e0, n in groups:
                        te.wait_ge(vsem, C[key])
                        if e0 == "h0" or e0 == "h1":
                            h = 0 if e0 == "h0" else 1
                            for j in range(n):
                                c0 = j * FD + h * 512
                                nc.tensor.matmul(
                                    pz[:, h * 512 : (h + 1) * 512], Ident,
                                    pslot(i)[:, c0 : c0 + 512],
                                    start=first_h[h], stop=False,
                                    skip_group_check=True,
                                )
                                first_h[h] = False
                            continue
                        for j in range(e0, e0 + n):
                            for h in (0, 1):
                                c0 = j * FD + h * 512
                                nc.tensor.matmul(
                                    pz[:, h * 512 : (h + 1) * 512], Ident,
                                    pslot(i)[:, c0 : c0 + 512],
                                    start=first_h[h], stop=False,
                                    skip_group_check=True,
                                )
                                first_h[h] = False
                    if i == 0:
                        te.wait_ge(obasem, 16)
                    elif i == 4:
                        te.wait_ge(obbsem, 16)
                    for h in (0, 1):
                        mm = nc.tensor.matmul(
                            pz[:, h * 512 : (h + 1) * 512], btsl(i), OHD,
                            start=False, stop=True, skip_group_check=True,
                        )
                        if h == 1:
                            mm.then_inc(zsem, 1)
                if i >= WLAG:
                    rr = i - WLAG
                    if WLPOOL:
                        te.wait_ge(psem, rr + 1)
                    else:
                        te.wait_ge(vsem, C[("wl", rr)])
                    wl = wlslot(rr)
                    nc.tensor.matmul(
                        ps0[:, :], OH3[:, rr, :], wl[:, 0:512],
                        start=(rr == 0), stop=False, skip_group_check=True,
                    )
                    nc.tensor.matmul(
                        ps1[:, :], OH3[:, rr, :], wl[:, 512:1024],
                        start=(rr == 0), stop=False, skip_group_check=True,
                    ).then_inc(msem, 1)
                    if rr == 0:
                        # accumulate the L*b_v rows early (order irrelevant)
                        nc.tensor.matmul(
                            ps0[:, :], lbvT, OHD,
                            start=False, stop=False, skip_group_check=True,
                        )
                        nc.tensor.matmul(
                            ps1[:, :], lbvT, OHD,
                            start=False, stop=False, skip_group_check=True,
                        )
            for rr in range(RC - WLAG, RC - NTAIL):
                if WLPOOL:
                    te.wait_ge(psem, rr + 1)
                else:
                    te.wait_ge(vsem, C[("wl", rr)])
                wl = wlslot(rr)
                nc.tensor.matmul(
                    ps0[:, :], OH3[:, rr, :], wl[:, 0:512],
                    start=False, stop=False, skip_group_check=True,
                )
                nc.tensor.matmul(
                    ps1[:, :], OH3[:, rr, :], wl[:, 512:1024],
                    start=False, stop=False, skip_group_check=True,
                ).then_inc(msem, 1)
            if NTAIL >= 2:
                rr = RC - 2
                te.wait_ge(vsem, C[("stt", rr)])
                wl = wlslot(rr)
                nc.tensor.matmul(
                    ps0[:, :], OH3[:, rr, :], wl[:, 0:512],
                    start=False, stop=False, skip_group_check=True,
                )
                nc.tensor.matmul(
                    ps1[:, :], OH3[:, rr, :], wl[:, 512:1024],
                    start=False, stop=False, skip_group_check=True,
                ).then_inc(msem, 1)
            rr = RC - 1
            wl = wlslot(rr)
            te.wait_ge(vsem, C[("stta", rr)])
            nc.tensor.matmul(
                ps0[:, :], OH3[:, rr, :], wl[:, 0:512],
                start=False, stop=True, skip_group_check=True,
            ).then_inc(msem, 1)
            te.wait_ge(vsem, C[("sttb", rr)])
            nc.tensor.matmul(
                ps1[:, :], OH3[:, rr, :], wl[:, 512:1024],
                start=False, stop=True, skip_group_check=True,
            ).then_inc(msem, 1)

    return nc


_CACHE = {}


def _buckets(a):
    """Deal r-indices into NK buckets of RC; zeros occupy each bucket's
    fast-mask positions first. Returns (buckets, nfast)."""
    zeros = [r for r in range(R) if a[r] == 0.0]
    pos = [r for r in range(R) if a[r] != 0.0]
    zbuck = [[] for _ in range(NK)]
    for j, r in enumerate(zeros):
        zbuck[j % NK].append(r)
    nfast = min(min(len(zb) for zb in zbuck), RC)
    mask = _fast_mask(nfast)
    pi = 0
    buckets = []
    for k in range(NK):
        zq = list(zbuck[k])
        rl = [None] * RC
        for i in range(RC):
            if mask[i]:
                rl[i] = zq.pop(0)
        for i in range(RC):
            if rl[i] is None:
                if zq:
                    rl[i] = zq.pop(0)
                else:
                    rl[i] = pos[pi]
                    pi += 1
        buckets.append(rl)
    return buckets, nfast


def _prepare(X, T, M, PD, alpha, w_v, w_t, b_t, b_v, ref_time):
    """Pack full inputs into per-core DRAM parameter maps.
    Returns (mask, buckets, in_maps)."""
    a = np.maximum(alpha.reshape(R), 0.0)
    s_ref = ref_time.reshape(R)
    nas = -(a * s_ref)
    bt4 = 4.0 * b_t[..., 0]              # [R, L, D]
    lbv = float(L) * b_v[:, 0, :]        # [R, D]

    buckets, nfast = _buckets(a)
    mask = _fast_mask(nfast)

    # per-r params: [w1|w0|w2|w3|wv] (5*D per iter)
    wts = np.stack(
        [w_t[..., 1], w_t[..., 0], w_t[..., 2], w_t[..., 3], w_v], axis=2
    )                                     # [R, L, 5, D]

    oh = np.zeros((L, RC, RC), np.float32)
    for r in range(RC):
        oh[:, r, r] = 1.0
    ident = np.eye(L, dtype=np.float32)
    ohd = np.zeros((64, 512), np.float32)
    for b in range(8):
        for d in range(64):
            ohd[d, b * 64 + d] = 1.0

    cstf = np.zeros((L, CONST_W), np.float32)
    cstf[:, 0 : RC * RC] = oh.reshape(L, RC * RC)
    cstf[:, RC * RC :] = ident
    cst = cstf.astype(_nbf16)

    in_maps = []
    for c in range(8):
        b0 = (c // NK) * BC
        rl = buckets[c % NK]
        tr = lambda x: np.ascontiguousarray(
            x[b0 : b0 + BC].transpose(1, 0, 2).reshape(L, FD)
        )
        cfc = np.zeros((L, CFC_W), np.float32)
        cfc[:, 0:RC] = a[rl]
        cfc[:, RC : 2 * RC] = nas[rl]
        cbf = np.zeros((L, CB_W), np.float32)
        trx = tr(X)
        cbf[:, XH0E : XH0E + 512] = trx[:, 0:512]
        cbf[:, XH1E : XH1E + 512] = trx[:, 512:FD]
        cbf[:, MTE : MTE + FD] = tr(M)
        cbf[:, PDE : PDE + FD] = tr(PD)
        for i, r in enumerate(rl):
            if i < 2:
                base = W01E + i * WS_W
            elif i < 6:
                base = W25E + (i - 2) * WS_W
            else:
                base = W611E + (i - 6) * WS_W
            cbf[:, base : base + WS_W] = wts[r].reshape(L, WS_W)
        oba = np.zeros((64, OBA_W), np.float32)
        oba[:, 0:512] = ohd
        obb = np.zeros((64, OBB_W), np.float32)
        for i, r in enumerate(rl):
            oba[0:D, 512 + i] = lbv[r]
            if i < 4:
                oba[0:D, 512 + 16 + i * L : 512 + 16 + (i + 1) * L] = bt4[r].T
            else:
                obb[0:D, (i - 4) * L : (i - 3) * L] = bt4[r].T
        in_maps.append(
            {
                "cfc": cfc,
                "cfT": tr(T).astype(_nbf16),
                "cb": np.ascontiguousarray(cbf).astype(_nbf16),
                "CST": cst,
                "OBA": oba.astype(_nbf16),
                "OBB": obb.astype(_nbf16),
            }
        )
    return mask, buckets, in_maps


def kernel(X, T, M, PD, alpha, w_v, w_t, b_t, b_v, ref_time):
    X = np.asarray(X, np.float32)
    T = np.asarray(T, np.float32)
    M = np.asarray(M, np.float32)
    PD = np.asarray(PD, np.float32)
    alpha = np.asarray(alpha, np.float32)
    w_v = np.asarray(w_v, np.float32)
    w_t = np.asarray(w_t, np.float32)
    b_t = np.asarray(b_t, np.float32)
    b_v = np.asarray(b_v, np.float32)
    ref_time = np.asarray(ref_time, np.float32)

    mask, buckets, in_maps = _prepare(
        X, T, M, PD, alpha, w_v, w_t, b_t, b_v, ref_time
    )

    if mask not in _CACHE:
        _CACHE[mask] = _build_graph(mask)
    nc = _CACHE[mask]

    trace = bool(os.environ.get("BASS_KERNEL_TRACE"))
    kw = {}
    if trace:
        tmpdir = os.environ.get("BASS_KERNEL_TRACE_DIR") or None
        kw = dict(trace=True, tmpdir=tmpdir)
    for _attempt in range(3):
        res = run_bass_kernel_spmd(nc, in_maps, core_ids=list(range(8)), **kw)
        outs = [
            np.asarray(res.results[c]["out"], np.float32) for c in range(8)
        ]
        if all(np.isfinite(o).all() for o in outs):
            break
    if trace:
        _CACHE["exec_time_ns"] = res.exec_time_ns
        print(f"HW exec time: {res.exec_time_ns} ns")

    out = np.zeros((B, R, D), np.float32)
    for c in range(8):
        b0 = (c // NK) * BC
        rl = buckets[c % NK]
        o = outs[c].reshape(RC, BC, D)
        for i, r in enumerate(rl):
            out[b0 : b0 + BC, r] = o[i]
    return out
